# revision 1
# baseline (speedup 1.0000x reference)
"""Trainium2 Bass kernel for nn_KPLoss_377957122199.

Keypoint loss = alpha*cross_entropy + beta*smoothL1(kp) + delta*smoothL1(Procrustes rot)
              + epsilon*smoothL1(centers),  alpha,beta,delta,eps = 1,4,5,6

Data-parallel over 8 NeuronCores: batch 8192 -> 1024 per core. Each core
produces per-partition partial sums; host combines (weighted means).

Key device tricks:
  * smooth_l1 sums via  sum f(d) = 0.5*sum d^2 - 0.5*sum u^2 + sum u - N/2,
    u = max(|d|,1)  (one tensor_scalar(abs_max)+accum and two ACT Square+accum)
  * cross entropy without max-subtraction (logits are O(5)); one-hot mask via
    gpsimd is_equal against an iota tile; sum(l_y) via fused tensor_tensor_reduce
  * Procrustes rotation R = polar(H) via Frobenius-scaled Newton iteration
    (4 iters + 1 Newton-Schulz polish), batched over all sections as
    [128,160] elementwise planes.
"""

import sys
for _p in ("/opt/trn_rl_repo", "/root/.axon_site/_ro/trn_rl_repo"):
    if _p not in sys.path:
        sys.path.insert(0, _p)

from contextlib import ExitStack

import numpy as np
import ml_dtypes

import concourse.bass as bass
import concourse.bacc as bacc
import concourse.mybir as mybir
import concourse.tile as tile
from concourse.bass_utils import run_bass_kernel_spmd

FP32 = mybir.dt.float32
BF16 = mybir.dt.bfloat16
AX = mybir.AxisListType
OP = mybir.AluOpType
AF = mybir.ActivationFunctionType

N_CORES = 8
B, K, NS, SEC = 8192, 400, 20, 20
S = K // SEC                      # 20 sections per sample
BC = B // N_CORES                 # 1024 samples per core
NCH_KP = BC // 128                # 8 keypoint chunks of 128 samples
SECS = BC * S                     # 20480 sections per core
SFD = SECS // 128                 # 160 sections per partition
NCH_CE = 20                       # cross-entropy chunks
TOK = BC * K                      # 409600 tokens per core
T_CE = TOK // (NCH_CE * 128)      # 160 tokens per partition per chunk
N_KP = BC * K * 3                 # smooth-l1 element count (kp and rot)
N_CENT = BC * S * 3

# acc column map (fp32 [128, NACC] output)
# smooth_l1 sums use the identity  sum f(d) = 0.5*(sum d^2 - sum relu(|d|-1)^2)
C_LSE = 0                         # NCH_CE cols
C_LY = C_LSE + NCH_CE
C_KP = C_LY + NCH_CE              # 2*NCH_KP cols: d2, r2 per chunk
C_ROT = C_KP + 2 * NCH_KP
C_CENT = C_ROT + 2 * NCH_KP
NACC = C_CENT + 2 * NCH_KP


def _emit(ctx: ExitStack, tc: "tile.TileContext", aps: dict):
    nc = tc.nc
    pk, gk, lg, lb, out = aps["pk"], aps["gk"], aps["lg"], aps["lb"], aps["out"]

    io = ctx.enter_context(tc.tile_pool(name="io", bufs=2))
    work = ctx.enter_context(tc.tile_pool(name="work", bufs=2))
    pers = ctx.enter_context(tc.tile_pool(name="pers", bufs=1))
    polp = ctx.enter_context(tc.tile_pool(name="polar", bufs=1))
    cep = ctx.enter_context(tc.tile_pool(name="ce", bufs=2))

    acc = pers.tile([128, NACC], FP32, tag="acc", name="acc")
    # every column is written exactly once by an accum_out; no memset needed
    neg1 = pers.tile([128, 1], FP32, tag="neg1", name="neg1")
    nc.gpsimd.memset(neg1[:], -1.0)

    sp_all = pers.tile([128, NCH_KP * 60], FP32, tag="sp", name="sp")   # per-chunk d-major point sums
    sg_all = pers.tile([128, NCH_KP * 60], FP32, tag="sg", name="sg")
    H = [[pers.tile([128, SFD], FP32, tag=f"H{i}{j}", name=f"H{i}{j}") for j in range(3)] for i in range(3)]

    # ---------------- phase 1: keypoint pass ----------------
    def kp_load_deint(c):
        pkc = io.tile([128, 1200], FP32, tag="pkc", name="pkc")
        gkc = io.tile([128, 1200], FP32, tag="gkc", name="gkc")
        nc.sync.dma_start(pkc[:], pk[c])
        nc.sync.dma_start(gkc[:], gk[c])
        pb = io.tile([128, 1200], BF16, tag="pb", name="pb")
        gb = io.tile([128, 1200], BF16, tag="gb", name="gb")
        # interleaved (s k d) -> d-major (d s k), cast to bf16
        for src, dst in ((pkc, pb), (gkc, gb)):
            v = src[:].rearrange("p (s k d) -> p d s k", s=SEC, k=SEC, d=3)
            for d in range(3):
                nc.gpsimd.tensor_copy(
                    dst[:, d * 400:(d + 1) * 400].rearrange("p (s k) -> p s k", s=SEC),
                    v[:, d],
                )
        return pb, gb

    def smooth_l1_acc(dt_tile, fd, col_base, c, u_tag):
        """sum d^2 and sum relu(|d|-1)^2 for this chunk -> two acc columns (ACT only)."""
        a = work.tile([128, fd], BF16, tag=u_tag, name=u_tag)
        nc.scalar.activation(a[:], dt_tile[:], AF.Abs)
        r = work.tile([128, fd], BF16, tag=u_tag + "r", name=u_tag + "r")
        nc.scalar.activation(r[:], a[:], AF.Relu, bias=neg1[:])
        tr = work.tile([128, fd], BF16, tag=u_tag + "tr", name=u_tag + "tr")
        nc.scalar.activation(tr[:], dt_tile[:], AF.Square,
                             accum_out=acc[:, col_base + c: col_base + c + 1])
        nc.scalar.activation(tr[:], r[:], AF.Square,
                             accum_out=acc[:, col_base + NCH_KP + c: col_base + NCH_KP + c + 1])

    for c in range(NCH_KP):
        pb, gb = kp_load_deint(c)
        # keypoint smooth-l1
        dt = work.tile([128, 1200], BF16, tag="kpd", name="kpd")
        nc.vector.tensor_sub(dt[:], pb[:], gb[:])
        smooth_l1_acc(dt, 1200, C_KP, c, "slu")
        # per-(d,section) point sums (sum over k): [128,3,20,20] -> [128,3,20]
        for src, dst in ((pb, sp_all), (gb, sg_all)):
            nc.vector.tensor_reduce(
                dst[:, c * 60:(c + 1) * 60].rearrange("p (d s) -> p d s", d=3),
                src[:].rearrange("p (d s k) -> p d s k", d=3, s=SEC, k=SEC),
                axis=AX.X, op=OP.add,
            )
        # center loss: mean diff = (sp-sg)/SEC
        dc = work.tile([128, 60], FP32, tag="centd", name="centd")
        nc.vector.tensor_sub(dc[:], sp_all[:, c * 60:(c + 1) * 60], sg_all[:, c * 60:(c + 1) * 60])
        dcm = work.tile([128, 60], BF16, tag="centdm", name="centdm")
        nc.vector.tensor_scalar(dcm[:], dc[:], 1.0 / SEC, None, OP.mult)
        smooth_l1_acc(dcm, 60, C_CENT, c, "slu")
        # raw H_ij = sum_k G_ki P_kj  (per section)
        eng = [nc.vector, nc.gpsimd]
        for i in range(3):
            for j in range(3):
                pr = work.tile([128, 400], BF16, tag=f"hprod{(i * 3 + j) % 2}", name=f"hprod{(i * 3 + j) % 2}")
                eng[(i * 3 + j) % 2].tensor_mul(
                    pr[:], gb[:, i * 400:(i + 1) * 400], pb[:, j * 400:(j + 1) * 400])
                nc.vector.tensor_reduce(
                    H[i][j][:, c * 20:(c + 1) * 20],
                    pr[:].rearrange("p (s k) -> p s k", s=SEC),
                    axis=AX.X, op=OP.add,
                )

    # H correction: H_ij -= (1/SEC) * sg_i * sp_j   (views over all chunks)
    sps = pers.tile([128, NCH_KP * 60], FP32, tag="sps", name="sps")
    nc.vector.tensor_scalar(sps[:], sp_all[:], 1.0 / SEC, None, OP.mult)

    def dsum_view(t, i):
        # [128, (chunk, d, s)] -> fixed d=i -> [128, chunk, s] == [128, SFD]
        return t[:].rearrange("p (c d s) -> p d c s", c=NCH_KP, d=3, s=S)[:, i]

    for i in range(3):
        for j in range(3):
            m = work.tile([128, SFD], FP32, tag="hc", name="hc")
            nc.vector.tensor_mul(m[:], dsum_view(sg_all, i), dsum_view(sps, j))
            nc.vector.tensor_sub(
                H[i][j][:].rearrange("p (c s) -> p c s", c=NCH_KP),
                H[i][j][:].rearrange("p (c s) -> p c s", c=NCH_KP),
                m[:].rearrange("p (c s) -> p c s", c=NCH_KP),
            )

    # ---------------- polar decomposition: R = polar(H) ----------------
    X = H  # in place; H not needed afterwards
    rr = [0]
    engs = [nc.vector, nc.gpsimd]

    def tt(op, out, a, b):
        engs[rr[0] % 2].tensor_tensor(out[:], a[:], b[:], op)
        rr[0] += 1

    def cof_det(Xc):
        C = [[polp.tile([128, SFD], FP32, tag=f"cof{i}{j}", name=f"cof{i}{j}") for j in range(3)] for i in range(3)]
        t1 = polp.tile([128, SFD], FP32, tag="cdt1", name="cdt1")
        idx = [(1, 2), (2, 0), (0, 1)]
        for i in range(3):
            for j in range(3):
                (a, b_), (cc, dd) = idx[i], idx[j]
                # cof[i][j] = X[a][cc]*X[b_][dd] - X[a][dd]*X[b_][cc]
                m1 = polp.tile([128, SFD], FP32, tag="cm1", name="cm1")
                m2 = polp.tile([128, SFD], FP32, tag="cm2", name="cm2")
                tt(OP.mult, m1, Xc[a][cc], Xc[b_][dd])
                tt(OP.mult, m2, Xc[a][dd], Xc[b_][cc])
                tt(OP.subtract, C[i][j], m1, m2)
        det = polp.tile([128, SFD], FP32, tag="det", name="det")
        nc.vector.tensor_mul(det[:], Xc[0][0][:], C[0][0][:])
        nc.vector.tensor_mul(t1[:], Xc[0][1][:], C[0][1][:])
        nc.vector.tensor_add(det[:], det[:], t1[:])
        nc.vector.tensor_mul(t1[:], Xc[0][2][:], C[0][2][:])
        nc.vector.tensor_add(det[:], det[:], t1[:])
        return C, det

    def frob2(M, tag):
        n2 = polp.tile([128, SFD], FP32, tag=tag)
        t = polp.tile([128, SFD], FP32, tag=tag + "t")
        nc.vector.tensor_mul(n2[:], M[0][0][:], M[0][0][:])
        for i in range(3):
            for j in range(3):
                if i == 0 and j == 0:
                    continue
                eng = engs[(i * 3 + j) % 2]
                eng.tensor_mul(t[:], M[i][j][:], M[i][j][:])
                nc.vector.tensor_add(n2[:], n2[:], t[:])
        return n2

    for it in range(4):
        C, det = cof_det(X)
        nX2 = frob2(X, "nx2")
        nC2 = frob2(C, "nc2")
        # zeta = (nC2/nX2)^(1/4) / sqrt(|det|)
        q = polp.tile([128, SFD], FP32, tag="q", name="q")
        qr = polp.tile([128, SFD], FP32, tag="qr", name="qr")
        nc.vector.reciprocal(qr[:], nX2[:])
        nc.vector.tensor_mul(q[:], nC2[:], qr[:])
        nc.scalar.activation(q[:], q[:], AF.Sqrt)
        nc.scalar.activation(q[:], q[:], AF.Sqrt)
        da = polp.tile([128, SFD], FP32, tag="da", name="da")
        nc.scalar.activation(da[:], det[:], AF.Abs)
        nc.scalar.activation(da[:], da[:], AF.Sqrt)
        dr = polp.tile([128, SFD], FP32, tag="dr", name="dr")
        nc.vector.reciprocal(dr[:], da[:])
        zeta = polp.tile([128, SFD], FP32, tag="zeta", name="zeta")
        nc.vector.tensor_mul(zeta[:], q[:], dr[:])
        # X' = 0.5*zeta*X + (0.5/(zeta*det)) * C
        hz = polp.tile([128, SFD], FP32, tag="hz", name="hz")
        nc.vector.tensor_scalar(hz[:], zeta[:], 0.5, None, OP.mult)
        u = polp.tile([128, SFD], FP32, tag="uu", name="uu")
        nc.vector.tensor_mul(u[:], zeta[:], det[:])
        w = polp.tile([128, SFD], FP32, tag="ww", name="ww")
        nc.vector.reciprocal(w[:], u[:])
        nc.vector.tensor_scalar(w[:], w[:], 0.5, None, OP.mult)
        Xn = [[polp.tile([128, SFD], FP32, tag=f"X{i}{j}", name=f"X{i}{j}") for j in range(3)] for i in range(3)]
        for i in range(3):
            for j in range(3):
                a = polp.tile([128, SFD], FP32, tag="ua", name="ua")
                b_ = polp.tile([128, SFD], FP32, tag="ub", name="ub")
                tt(OP.mult, a, X[i][j], hz)
                tt(OP.mult, b_, C[i][j], w)
                tt(OP.add, Xn[i][j], a, b_)
        X = Xn

    # one Newton-Schulz polish: X = X(1.5 I - 0.5 X^T X)
    Y = [[None] * 3 for _ in range(3)]
    for i in range(3):
        for j in range(i, 3):
            y = polp.tile([128, SFD], FP32, tag=f"Y{i}{j}", name=f"Y{i}{j}")
            t = polp.tile([128, SFD], FP32, tag="yt", name="yt")
            nc.vector.tensor_mul(y[:], X[0][i][:], X[0][j][:])
            for k in (1, 2):
                engs[k % 2].tensor_mul(t[:], X[k][i][:], X[k][j][:])
                nc.vector.tensor_add(y[:], y[:], t[:])
            Y[i][j] = Y[j][i] = y
    W = [[None] * 3 for _ in range(3)]
    for i in range(3):
        for j in range(i, 3):
            w_ = polp.tile([128, SFD], FP32, tag=f"W{i}{j}", name=f"W{i}{j}")
            if i == j:
                nc.vector.tensor_scalar(w_[:], Y[i][j][:], -0.5, 1.5, OP.mult, OP.add)
            else:
                nc.vector.tensor_scalar(w_[:], Y[i][j][:], -0.5, None, OP.mult)
            W[i][j] = W[j][i] = w_
    R = [[polp.tile([128, SFD], FP32, tag=f"R{i}{j}", name=f"R{i}{j}") for j in range(3)] for i in range(3)]
    for i in range(3):
        for j in range(3):
            t = polp.tile([128, SFD], FP32, tag="rt", name="rt")
            nc.vector.tensor_mul(R[i][j][:], X[i][0][:], W[0][j][:])
            for k in (1, 2):
                engs[k % 2].tensor_mul(t[:], X[i][k][:], W[k][j][:])
                nc.vector.tensor_add(R[i][j][:], R[i][j][:], t[:])

    # v_j (per section) = (1/SEC) * (sum_i sp_i R_ij - sg_j)
    v_all = pers.tile([128, 3 * SFD], FP32, tag="vall", name="vall")
    for j in range(3):
        vj = v_all[:, j * SFD:(j + 1) * SFD]
        t = work.tile([128, SFD], FP32, tag="vt", name="vt")
        nc.vector.tensor_mul(vj, dsum_view(sp_all, 0), R[0][j][:])
        for i in (1, 2):
            engs[i % 2].tensor_mul(t[:], dsum_view(sp_all, i), R[i][j][:])
            nc.vector.tensor_add(vj, vj, t[:])
        nc.vector.tensor_sub(vj, vj, dsum_view(sg_all, j))
        nc.vector.tensor_scalar(vj, vj, 1.0 / SEC, None, OP.mult)

    # ---------------- phase 3: rotation residual ----------------
    for c in range(NCH_KP):
        pb, gb = kp_load_deint(c)
        # expand per-section R, v over k (broadcast) in bf16
        Re = [[work.tile([128, 400], BF16, tag=f"Re{i}{j}", name=f"Re{i}{j}", bufs=1) for j in range(3)] for i in range(3)]
        for i in range(3):
            for j in range(3):
                nc.gpsimd.tensor_copy(
                    Re[i][j][:].rearrange("p (s k) -> p s k", s=S),
                    R[i][j][:, c * 20:(c + 1) * 20].unsqueeze(2).broadcast_to([128, S, SEC]),
                )
        rfull = work.tile([128, 1200], BF16, tag="rfull", name="rfull")
        for j in range(3):
            ve = work.tile([128, 400], BF16, tag="ve", name="ve")
            nc.gpsimd.tensor_copy(
                ve[:].rearrange("p (s k) -> p s k", s=S),
                v_all[:, j * SFD + c * 20: j * SFD + (c + 1) * 20]
                .unsqueeze(2).broadcast_to([128, S, SEC]),
            )
            rj = rfull[:, j * 400:(j + 1) * 400]
            t = work.tile([128, 400], BF16, tag="rt3", name="rt3")
            nc.vector.tensor_mul(rj, pb[:, 0:400], Re[0][j][:])
            for i in (1, 2):
                nc.vector.tensor_mul(t[:], pb[:, i * 400:(i + 1) * 400], Re[i][j][:])
                nc.vector.tensor_add(rj, rj, t[:])
            nc.vector.tensor_sub(rj, rj, gb[:, j * 400:(j + 1) * 400])
            nc.vector.tensor_sub(rj, rj, ve[:])
        smooth_l1_acc(rfull, 1200, C_ROT, c, "slu")

    # ---------------- cross entropy ----------------
    iota = pers.tile([128, T_CE * NS], BF16, tag="iota", name="iota")
    nc.gpsimd.iota(iota[:], pattern=[[0, T_CE], [1, NS]], base=0,
                   channel_multiplier=0, allow_small_or_imprecise_dtypes=True)
    for c in range(NCH_CE):
        lgc = cep.tile([128, T_CE * NS], FP32, tag="lgc", name="lgc")
        nc.sync.dma_start(lgc[:], lg[c])
        lbc = cep.tile([128, T_CE], BF16, tag="lbc", name="lbc")
        nc.sync.dma_start(lbc[:], lb[c])
        ex = cep.tile([128, T_CE * NS], BF16, tag="ex", name="ex")
        nc.scalar.activation(ex[:], lgc[:], AF.Exp)
        s10 = cep.tile([128, T_CE * 10], BF16, tag="s10", name="s10")
        ex3 = ex[:].rearrange("p (t n) -> p t n", t=T_CE)
        nc.vector.tensor_add(
            s10[:].rearrange("p (t n) -> p t n", t=T_CE),
            ex3[:, :, 0:10], ex3[:, :, 10:20])
        se = cep.tile([128, T_CE], FP32, tag="se", name="se")
        nc.vector.tensor_reduce(
            se[:], s10[:].rearrange("p (t n) -> p t n", t=T_CE), axis=AX.X, op=OP.add)
        lt = cep.tile([128, T_CE], BF16, tag="lt", name="lt")
        nc.scalar.activation(lt[:], se[:], AF.Ln,
                             accum_out=acc[:, C_LSE + c: C_LSE + c + 1])
        lbe = cep.tile([128, T_CE * NS], BF16, tag="lbe", name="lbe", bufs=1)
        nc.gpsimd.tensor_copy(
            lbe[:].rearrange("p (t n) -> p t n", t=T_CE),
            lbc[:].unsqueeze(2).broadcast_to([128, T_CE, NS]))
        mask = cep.tile([128, T_CE * NS], BF16, tag="mask", name="mask", bufs=1)
        nc.vector.tensor_tensor(mask[:], lbe[:], iota[:], OP.is_equal)
        # sum l_y = sum ln(sum_j mask * exp(l))  (masked-exp keeps 2x bf16 modes)
        me = cep.tile([128, T_CE * NS], BF16, tag="me", name="me", bufs=1)
        nc.vector.tensor_mul(me[:], mask[:], ex[:])
        m10 = cep.tile([128, T_CE * 10], BF16, tag="m10", name="m10")
        me3 = me[:].rearrange("p (t n) -> p t n", t=T_CE)
        nc.vector.tensor_add(
            m10[:].rearrange("p (t n) -> p t n", t=T_CE),
            me3[:, :, 0:10], me3[:, :, 10:20])
        mse = cep.tile([128, T_CE], FP32, tag="mse", name="mse")
        nc.vector.tensor_reduce(
            mse[:], m10[:].rearrange("p (t n) -> p t n", t=T_CE), axis=AX.X, op=OP.add)
        lt2 = cep.tile([128, T_CE], BF16, tag="lt2", name="lt2")
        nc.scalar.activation(lt2[:], mse[:], AF.Ln,
                             accum_out=acc[:, C_LY + c: C_LY + c + 1])

    nc.sync.dma_start(out[:], acc[:])


_CACHE = {}


def _build():
    if "nc" in _CACHE:
        return _CACHE["nc"]
    nc = bacc.Bacc("TRN2", target_bir_lowering=False, debug=False,
                   enable_asserts=False, num_devices=N_CORES)
    aps = {
        "pk": nc.dram_tensor("pk", [NCH_KP, 128, 1200], FP32, kind="ExternalInput").ap(),
        "gk": nc.dram_tensor("gk", [NCH_KP, 128, 1200], FP32, kind="ExternalInput").ap(),
        "lg": nc.dram_tensor("lg", [NCH_CE, 128, T_CE * NS], FP32, kind="ExternalInput").ap(),
        "lb": nc.dram_tensor("lb", [NCH_CE, 128, T_CE], BF16, kind="ExternalInput").ap(),
        "out": nc.dram_tensor("out", [128, NACC], FP32, kind="ExternalOutput").ap(),
    }
    with tile.TileContext(nc) as tc:
        with ExitStack() as ctx:
            _emit(ctx, tc, aps)
    nc.compile()
    _CACHE["nc"] = nc
    return nc


def _shard_inputs(pred_keypoints, gt_keypoints, pred_section_logits, gt_section_label):
    pk = np.ascontiguousarray(pred_keypoints, dtype=np.float32).reshape(N_CORES, NCH_KP, 128, 1200)
    gk = np.ascontiguousarray(gt_keypoints, dtype=np.float32).reshape(N_CORES, NCH_KP, 128, 1200)
    lg = np.ascontiguousarray(pred_section_logits, dtype=np.float32).reshape(
        N_CORES, NCH_CE, 128, T_CE * NS)
    lb = np.ascontiguousarray(gt_section_label).reshape(N_CORES, NCH_CE, 128, T_CE).astype(
        ml_dtypes.bfloat16)
    return [
        {"pk": pk[i], "gk": gk[i], "lg": lg[i], "lb": lb[i]}
        for i in range(N_CORES)
    ]


def combine_accs(accs):
    """accs: list of [128, NACC] fp32 arrays (one per core) -> scalar loss."""
    tot = np.zeros(NACC, dtype=np.float64)
    for a in accs:
        tot += a.astype(np.float64).sum(axis=0)

    def sl1(base, n_per_chunk_elems):
        d2 = tot[base:base + NCH_KP].sum()
        r2 = tot[base + NCH_KP:base + 2 * NCH_KP].sum()
        return 0.5 * (d2 - r2)

    ce_sum = tot[C_LSE:C_LSE + NCH_CE].sum() - tot[C_LY:C_LY + NCH_CE].sum()
    kp_sum = sl1(C_KP, N_KP)
    rot_sum = sl1(C_ROT, N_KP)
    cent_sum = sl1(C_CENT, N_CENT)
    total = (1.0 * ce_sum / (B * K)
             + 4.0 * kp_sum / (B * K * 3)
             + 5.0 * rot_sum / (B * K * 3)
             + 6.0 * cent_sum / (B * S * 3))
    return np.float32(total)


def kernel(**inputs) -> np.ndarray:
    nc = _build()
    in_maps = _shard_inputs(**inputs)
    res = run_bass_kernel_spmd(nc, in_maps, list(range(N_CORES))).results
    return combine_accs([res[i]["out"] for i in range(N_CORES)])



# revision 18
# speedup vs baseline: 2.9241x; 2.9241x over previous
"""Trainium2 Bass kernel for nn_KPLoss_377957122199 (v2, engine-rebalanced).

loss = 1*CE + 4*smoothL1(kp) + 5*smoothL1(Procrustes rot residual)
     + 6*smoothL1(section-center diff)

Data-parallel over 8 cores (batch 8192 -> 1024/core). Key design:
  * custom DVE ops: SL1_DIFF (fused smooth-L1 sum of (in0-in1) in one
    vector op via sum f(d) = sum (d - 0.5*clamp(d))*clamp(d)), and
    ONEHOT_DOT (sum_t logits[y_t, t] via PageIdx compare, one op/chunk)
  * CE sum-of-exp on the idle TensorEngine: logits shipped twice
    (n-major [128,(n,t)] for ONEHOT; flat-transposed [100,4096] so a
    block-ones matmul reduces NS=20 on partitions into PSUM), ln reads
    PSUM packed 4 chunks/ACT via matmul tile_position.
  * keypoints host-deinterleaved to [d,k,s] bf16; kept in SBUF across
    both passes; H products/reductions and the rotation residual use
    stride-0 broadcast views (no materialized broadcasts).
  * batched 3x3 polar: 9 components contiguous [128,9*160] bf16,
    cofactors via shifted views of a 6x6-duplicated tile, scaled-Newton
    x3 + one Newton-Schulz polish, guarded (|det| clamp + Sign).
  * single ACT table set (exp/ln/sign only) - one table load.
"""

import sys
for _p in ("/opt/trn_rl_repo", "/root/.axon_site/_ro/trn_rl_repo"):
    if _p not in sys.path:
        sys.path.insert(0, _p)

import os
from contextlib import ExitStack
from operator import add as _add_op

import numpy as np
import ml_dtypes

import concourse.bass as bass
import concourse.bacc as bacc
import concourse.mybir as mybir
import concourse.tile as tile
from concourse.bass_utils import run_bass_kernel_spmd

# ---- custom DVE ops (registered at import) --------------------------------
import concourse.dve_ops as dve_ops
from concourse.dve_ops import DveOp, OPS
from concourse.dve_spec import (
    C0, C1, C2, PageIdx, Spec, Src0, Src1, Zero,
    _has_src1, eq, lower, maxx, minn, select,
)
from concourse.dve_uop import DveOpSpec


def _sl1_ref(in0, in1, s0, s1, imm2):
    d = in0.astype(np.float32) - in1.astype(np.float32)
    t = np.clip(d, s0, s1)
    return (d - imm2 * t) * t


def _oh_ref(in0, in1, s0, s1, imm2):
    raise NotImplementedError


def _register(name, spec, subdim):
    if name in dve_ops._SUB_OPCODE_FOR_NAME:
        return next(o for o in OPS if o.name == name)
    row = dve_ops._CUSTOM_DVE_ROW_BASE + len(OPS)
    assert row < 0x20
    op = DveOp(name, spec, subdim=subdim, uops_sha={})
    for ver in ("v3", "v4"):
        s = DveOpSpec(name=name, opcode=row, uops=lower(spec, ver=ver),
                      rd1_en=_has_src1(spec))
        op.uops_sha[ver] = s.sha(ver)
    OPS.append(op)
    dve_ops._SUB_OPCODE_FOR_NAME[name] = row
    return op


_d = Src0 - Src1
_t = minn(maxx(_d, C0), C1)
SL1_DIFF = _register("SL1_DIFF", Spec(body=(_d - _t * C2) * _t, accum=_add_op,
                                      reference=_sl1_ref), subdim=False)
_pg = PageIdx(C0, C1)
ONEHOT_DOT = _register("ONEHOT_DOT",
                       Spec(body=select(eq(Src1, _pg), Src0, Zero),
                            accum=_add_op, reference=_oh_ref), subdim=True)

FP32 = mybir.dt.float32
BF16 = mybir.dt.bfloat16
AX = mybir.AxisListType
OP = mybir.AluOpType
AF = mybir.ActivationFunctionType

N_CORES = 8
B, K, NS, SEC = 8192, 400, 20, 20
S = K // SEC                    # 20 sections / sample
BC = B // N_CORES               # 1024 samples / core
NCH_KP = BC // 128              # 8 keypoint chunks
SFD = NCH_KP * S                # 160 sections per partition
NCH_CE = 20                     # CE chunks
TOKC = BC * K // NCH_CE         # 20480 tokens / CE chunk
T_CE = TOKC // 128              # 160 tokens / partition (n-major layout)
FFL = TOKC * NS // 100          # 4096 cols in flat [100, .] layout

N_ITER = 3                      # polar Newton iterations

# acc column map
C_LSE = 0                       # 5 cols (groups of 4 chunks; rows 32q+0..4)
C_LY = C_LSE + 5                # 20 cols
C_KP = C_LY + NCH_CE            # 8
C_ROT = C_KP + NCH_KP           # 8
C_CENT = C_ROT + NCH_KP         # 1
NACC = C_CENT + 1


def _emit(ctx, tc, aps):
    nc = tc.nc
    pk, gk, lgn, lgf, lb, ob, out = (aps[k] for k in
                                     ("pk", "gk", "lgn", "lgf", "lb", "ob", "out"))

    pers = ctx.enter_context(tc.tile_pool(name="pers", bufs=1))
    scr = ctx.enter_context(tc.tile_pool(name="scr", bufs=1))
    cep = ctx.enter_context(tc.tile_pool(name="ce", bufs=2))
    psp = ctx.enter_context(tc.tile_pool(name="ps", bufs=1, space="PSUM"))

    acc = pers.tile([128, NACC], FP32, tag="acc", name="acc")
    oneblk = pers.tile([100, 5], BF16, tag="oneblk", name="oneblk")
    nc.sync.dma_start(oneblk[:], ob)
    lnhalf = pers.tile([128, 1], FP32, tag="lnhalf", name="lnhalf")
    nc.gpsimd.memset(lnhalf[:], float(np.log(0.5)))

    # keypoint chunks persist across phase 1 and 3
    pb = [pers.tile([128, 1200], BF16, tag=f"pb{c}", name=f"pb{c}") for c in range(NCH_KP)]
    gb = [pers.tile([128, 1200], BF16, tag=f"gb{c}", name=f"gb{c}") for c in range(NCH_KP)]
    for c in range(NCH_KP):
        nc.sync.dma_start(pb[c][:], pk[c])
        nc.sync.dma_start(gb[c][:], gk[c])

    # ---------------- cross entropy ----------------
    psum = psp.tile([128, FFL], FP32, tag="mm", name="mm")
    for c in range(NCH_CE):
        lgnc = cep.tile([128, NS * T_CE], BF16, tag="lgn", name="lgn")
        nc.sync.dma_start(lgnc[:], lgn[c])
        lbc = cep.tile([128, T_CE], BF16, tag="lbc", name="lbc")
        nc.sync.dma_start(lbc[:], lb[c])
        lgfc = cep.tile([100, FFL], BF16, tag="lgf", name="lgf")
        nc.sync.dma_start(lgfc[:], lgf[c])

        # l_y: one custom op
        dmp = scr.tile([128, NS * T_CE], BF16, tag="dmp", name="dmp")
        nc.vector._custom_dve(
            ONEHOT_DOT,
            out=dmp[:].rearrange("p (n t) -> p n t", n=NS),
            in0=lgnc[:].rearrange("p (n t) -> p n t", n=NS),
            in1=lbc[:].unsqueeze(1).broadcast_to([128, NS, T_CE]),
            s0=0.0, s1=1.0, accum_out=acc[:, C_LY + c:C_LY + c + 1])

        # lse: exp (scalar) -> block-ones matmul (PE) -> ln on packed PSUM
        ex = lgfc
        nc.scalar.activation(ex[:], lgfc[:], AF.Exp)
        q = c % 4
        for h in range(FFL // 512):
            nc.tensor.matmul(
                psum[32 * q:32 * q + 5, h * 512:(h + 1) * 512],
                oneblk[:], ex[:, h * 512:(h + 1) * 512],
                start=True, stop=True, tile_position=(0, 32 * q))
        if q == 3:
            g = c // 4
            lnd = scr.tile([101, FFL], BF16, tag="lnd", name="lnd")
            nc.scalar.activation(lnd[:], psum[0:101, :], AF.Ln,
                                 accum_out=acc[0:101, C_LSE + g:C_LSE + g + 1])

    # ---------------- phase 1: keypoints ----------------
    H = pers.tile([128, 9 * SFD], BF16, tag="H", name="H")
    sp = pers.tile([128, 3 * SFD], BF16, tag="sp", name="sp")
    sg = pers.tile([128, 3 * SFD], BF16, tag="sg", name="sg")

    wk1 = tc.alloc_tile_pool(name="wk1", bufs=2)
    work = wk1
    for c in range(NCH_KP):
        p3 = pb[c][:].rearrange("p (d f) -> p d f", d=3)        # [128,3,400]
        g3 = gb[c][:].rearrange("p (d f) -> p d f", d=3)
        dmp = scr.tile([128, 1200], BF16, tag="dump1200", name="dump1200")
        nc.vector._custom_dve(SL1_DIFF, out=dmp[:], in0=pb[c][:], in1=gb[c][:],
                              s0=-1.0, s1=1.0, imm2=0.5,
                              accum_out=acc[:, C_KP + c:C_KP + c + 1])
        # H products: T[m=(i,j)] = g_i * p_j over (k,s)
        T = work.tile([128, 9 * 400], BF16, tag="hT", name="hT")
        nc.vector.tensor_tensor(
            T[:].rearrange("p (i j f) -> p i j f", i=3, j=3),
            g3.unsqueeze(2).broadcast_to([128, 3, 3, 400]),
            p3.unsqueeze(1).broadcast_to([128, 3, 3, 400]), OP.mult)
        # k-tree: 20 -> 10 -> 5 -> reduce
        T4 = T[:].rearrange("p (m k s) -> p m k s", m=9, k=SEC)
        A1 = work.tile([128, 9 * 10 * S], BF16, tag="hA1", name="hA1")
        A1v = A1[:].rearrange("p (m k s) -> p m k s", m=9, k=10, s=S)
        nc.vector.tensor_tensor(A1v, T4[:, :, 0:10], T4[:, :, 10:20], OP.add)
        A2 = work.tile([128, 9 * 5 * S], BF16, tag="hA2", name="hA2")
        A2v = A2[:].rearrange("p (m k s) -> p m k s", m=9, k=5, s=S)
        nc.vector.tensor_tensor(A2v, A1v[:, :, 0:5], A1v[:, :, 5:10], OP.add)
        nc.vector.tensor_reduce(
            H[:].rearrange("p (m f) -> p m f", m=9)[:, :, c * S:(c + 1) * S],
            A2[:].rearrange("p (m k s) -> p m s k", m=9, k=5, s=S),
            axis=AX.X, op=OP.add)
        # point sums over k (gpsimd to offload vector)
        for src, dst in ((p3, sp), (g3, sg)):
            s4 = src.rearrange("p d (k s) -> p d k s", k=SEC)
            B1 = work.tile([128, 3 * 10 * S], BF16, tag="sB1", name="sB1")
            B1v = B1[:].rearrange("p (d k s) -> p d k s", d=3, k=10, s=S)
            nc.gpsimd.tensor_tensor(B1v, s4[:, :, 0:10], s4[:, :, 10:20], OP.add)
            B2 = work.tile([128, 3 * 5 * S], BF16, tag="sB2", name="sB2")
            B2v = B2[:].rearrange("p (d k s) -> p d k s", d=3, k=5, s=S)
            nc.gpsimd.tensor_tensor(B2v, B1v[:, :, 0:5], B1v[:, :, 5:10], OP.add)
            C1t = work.tile([128, 3 * 2 * S], BF16, tag="sC1", name="sC1")
            C1v = C1t[:].rearrange("p (d k s) -> p d k s", d=3, k=2, s=S)
            nc.gpsimd.tensor_tensor(C1v, B2v[:, :, 0:2], B2v[:, :, 2:4], OP.add)
            C2t = work.tile([128, 3 * S], BF16, tag="sC2", name="sC2")
            C2v = C2t[:].rearrange("p (d s) -> p d s", d=3)
            nc.gpsimd.tensor_tensor(C2v, C1v[:, :, 0], C1v[:, :, 1], OP.add)
            nc.gpsimd.tensor_tensor(
                dst[:].rearrange("p (d f) -> p d f", d=3)[:, :, c * S:(c + 1) * S],
                C2v, B2v[:, :, 4], OP.add)

    wk1.release()
    # center loss: smoothL1((sp-sg)/SEC) over [128, 3*SFD]
    sps = pers.tile([128, 3 * SFD], BF16, tag="sps", name="sps")
    sgs = pers.tile([128, 3 * SFD], BF16, tag="sgs", name="sgs")
    nc.vector.tensor_scalar(sps[:], sp[:], 1.0 / SEC, None, OP.mult)
    nc.vector.tensor_scalar(sgs[:], sg[:], 1.0 / SEC, None, OP.mult)
    dmpc = scr.tile([128, 3 * SFD], BF16, tag="dmpc", name="dmpc")
    nc.vector._custom_dve(SL1_DIFF, out=dmpc[:], in0=sps[:], in1=sgs[:],
                          s0=-1.0, s1=1.0, imm2=0.5,
                          accum_out=acc[:, C_CENT:C_CENT + 1])

    # H -= sg_i * sp_j / SEC
    sp3 = sp[:].rearrange("p (d f) -> p d f", d=3)
    sg3 = sg[:].rearrange("p (d f) -> p d f", d=3)
    M = scr.tile([128, 9 * SFD], BF16, tag="hcM", name="hcM")
    nc.vector.tensor_tensor(
        M[:].rearrange("p (i j f) -> p i j f", i=3, j=3),
        sg3.unsqueeze(2).broadcast_to([128, 3, 3, SFD]),
        sp3.unsqueeze(1).broadcast_to([128, 3, 3, SFD]), OP.mult)
    nc.vector.tensor_scalar(M[:], M[:], 1.0 / SEC, None, OP.mult)
    nc.vector.tensor_tensor(H[:], H[:], M[:], OP.subtract)

    # ---------------- polar decomposition (batched 3x3, bf16) ----------------
    pol = tc.alloc_tile_pool(name="pol", bufs=1)
    A66 = pol.tile([128, 36 * SFD], BF16, tag="A66", name="A66")
    A = A66[:].rearrange("p (a b f) -> p a b f", a=6, b=6)
    X = A[:, 0:3, 0:3]                                  # X lives inside A66
    H4 = H[:].rearrange("p (i j f) -> p i j f", i=3, j=3)
    nc.vector.tensor_copy(X, H4)
    Cf = pol.tile([128, 9 * SFD], BF16, tag="cof", name="cof")
    C3v = Cf[:].rearrange("p (i j f) -> p i j f", i=3, j=3)
    SX = pol.tile([128, 9 * SFD], BF16, tag="sqX", name="sqX")
    det = pol.tile([128, SFD], BF16, tag="det", name="det")
    t160a = pol.tile([128, SFD], FP32, tag="t160a", name="t160a")
    t160b = pol.tile([128, SFD], FP32, tag="t160b", name="t160b")
    adet = pol.tile([128, SFD], FP32, tag="adet", name="adet")
    sgn = pol.tile([128, SFD], BF16, tag="sgn", name="sgn")
    nx2 = pol.tile([128, SFD], BF16, tag="nx2", name="nx2")
    nc2_ = pol.tile([128, SFD], BF16, tag="nc2", name="nc2")
    zln = pol.tile([128, SFD], FP32, tag="zln", name="zln")
    hz = pol.tile([128, SFD], BF16, tag="hz", name="hz")
    wz = pol.tile([128, SFD], BF16, tag="wz", name="wz")
    wf = pol.tile([128, SFD], FP32, tag="wf", name="wf")

    def frob(dst, src4):
        sxw = SX[:].rearrange("p (i j f) -> p i j f", i=3, j=3)
        nc.vector.tensor_tensor(sxw, src4, src4, OP.mult)
        sx = SX[:].rearrange("p (m f) -> p m f", m=9)
        q1 = pol.tile([128, 4 * SFD], BF16, tag="fq1", name="fq1")
        q1v = q1[:].rearrange("p (m f) -> p m f", m=4)
        nc.vector.tensor_tensor(q1v, sx[:, 0:4], sx[:, 4:8], OP.add)
        q2 = pol.tile([128, 2 * SFD], BF16, tag="fq2", name="fq2")
        q2v = q2[:].rearrange("p (m f) -> p m f", m=2)
        nc.vector.tensor_tensor(q2v, q1v[:, 0:2], q1v[:, 2:4], OP.add)
        nc.vector.tensor_tensor(dst.unsqueeze(1), q2v[:, 0:1], q2v[:, 1:2], OP.add)
        nc.vector.tensor_tensor(dst, dst, sx[:, 8], OP.add)

    for it in range(N_ITER):
        # duplicate X -> A66 quadrants
        nc.vector.tensor_copy(A[:, 0:3, 3:6], X)
        nc.vector.tensor_copy(A[:, 3:6, :], A[:, 0:3, :])
        # cofactors: C[i][j] = A[i+1][j+1]A[i+2][j+2] - A[i+1][j+2]A[i+2][j+1]
        T1 = pol.tile([128, 9 * SFD], BF16, tag="ct1", name="ct1")
        nc.vector.tensor_tensor(
            T1[:].rearrange("p (i j f) -> p i j f", i=3, j=3),
            A[:, 1:4, 1:4], A[:, 2:5, 2:5], OP.mult)
        T2 = pol.tile([128, 9 * SFD], BF16, tag="ct2", name="ct2")
        nc.vector.tensor_tensor(
            T2[:].rearrange("p (i j f) -> p i j f", i=3, j=3),
            A[:, 1:4, 2:5], A[:, 2:5, 1:4], OP.mult)
        nc.vector.tensor_tensor(Cf[:], T1[:], T2[:], OP.subtract)
        # det = sum_j X[0][j] * C[0][j]
        P0 = pol.tile([128, 3 * SFD], BF16, tag="dp0", name="dp0")
        P0v = P0[:].rearrange("p (j f) -> p j f", j=3)
        nc.vector.tensor_tensor(P0v, X[:, 0], C3v[:, 0], OP.mult)
        nc.vector.tensor_tensor(det[:].unsqueeze(1), P0v[:, 0:1], P0v[:, 1:2], OP.add)
        nc.vector.tensor_tensor(det[:], det[:], P0v[:, 2], OP.add)
        # guards + zeta = exp(0.25 ln(nC2/nX2) - 0.5 ln|det|)
        frob(nx2[:], X)
        frob(nc2_[:], C3v)
        nc.vector.tensor_scalar(nx2[:], nx2[:], 1e-12, None, OP.max)
        nc.vector.tensor_scalar(nc2_[:], nc2_[:], 1e-12, None, OP.max)
        nc.scalar.activation(adet[:], det[:], AF.Abs)
        nc.vector.tensor_scalar(adet[:], adet[:], 1e-6, None, OP.max)
        nc.vector.tensor_scalar(sgn[:], det[:], 0.0, None, OP.is_ge)
        nc.vector.tensor_scalar(sgn[:], sgn[:], 2.0, -1.0, OP.mult, OP.add)
        nc.scalar.activation(t160a[:], nc2_[:], AF.Ln)
        nc.scalar.activation(t160b[:], nx2[:], AF.Ln)
        nc.vector.tensor_tensor(t160a[:], t160a[:], t160b[:], OP.subtract)
        nc.scalar.activation(t160b[:], adet[:], AF.Ln)
        nc.vector.tensor_scalar(t160b[:], t160b[:], -2.0, None, OP.mult)
        nc.vector.tensor_tensor(zln[:], t160a[:], t160b[:], OP.add)
        nc.vector.tensor_scalar(zln[:], zln[:], 0.25, None, OP.mult)
        # hz = 0.5*zeta = exp(zln + ln 0.5); w = 0.5*sgn/(zeta*|det|)
        nc.scalar.activation(hz[:], zln[:], AF.Exp, bias=lnhalf[:])
        nc.scalar.activation(t160a[:], zln[:], AF.Exp)
        nc.vector.tensor_tensor(t160b[:], t160a[:], adet[:], OP.mult)
        nc.vector.tensor_scalar(t160b[:], t160b[:], 2.0, None, OP.mult)
        nc.vector.reciprocal_approx_fast(wf[:], t160b[:])
        nc.vector.tensor_tensor(wz[:], wf[:], sgn[:], OP.mult)
        # X = X*hz + C*w  (broadcast over 9 components)
        hzb = hz[:].unsqueeze(1).unsqueeze(1).broadcast_to([128, 3, 3, SFD])
        wzb = wz[:].unsqueeze(1).unsqueeze(1).broadcast_to([128, 3, 3, SFD])
        U1 = pol.tile([128, 9 * SFD], BF16, tag="u1", name="u1")
        U1v = U1[:].rearrange("p (i j f) -> p i j f", i=3, j=3)
        nc.vector.tensor_tensor(U1v, X, hzb, OP.mult)
        U2 = pol.tile([128, 9 * SFD], BF16, tag="u2", name="u2")
        U2v = U2[:].rearrange("p (i j f) -> p i j f", i=3, j=3)
        nc.vector.tensor_tensor(U2v, C3v, wzb, OP.mult)
        nc.vector.tensor_tensor(X, U1v, U2v, OP.add)

    # Newton-Schulz polish: R = X (1.5 I - 0.5 X^T X)
    Y = pol.tile([128, 9 * SFD], BF16, tag="Y", name="Y")
    Yv = Y[:].rearrange("p (i j f) -> p i j f", i=3, j=3)
    Tk = pol.tile([128, 9 * SFD], BF16, tag="Tk", name="Tk")
    Tkv = Tk[:].rearrange("p (i j f) -> p i j f", i=3, j=3)
    for k in range(3):
        xk = A[:, k, 0:3]                               # [128, 3, SFD] = X[k][*]
        dst = Yv if k == 0 else Tkv
        nc.vector.tensor_tensor(
            dst, xk.unsqueeze(2).broadcast_to([128, 3, 3, SFD]),
            xk.unsqueeze(1).broadcast_to([128, 3, 3, SFD]), OP.mult)
        if k:
            nc.vector.tensor_tensor(Y[:], Y[:], Tk[:], OP.add)
    W = pol.tile([128, 9 * SFD], BF16, tag="W", name="W")
    nc.vector.tensor_scalar(W[:], Y[:], -0.5, None, OP.mult)
    Wv = W[:].rearrange("p (m f) -> p m f", m=9)
    for m in (0, 4, 8):
        nc.vector.tensor_scalar(Wv[:, m], Wv[:, m], 1.5, None, OP.add)
    R = pers.tile([128, 9 * SFD], BF16, tag="R", name="R")
    Rv = R[:].rearrange("p (i j f) -> p i j f", i=3, j=3)
    Wv4 = W[:].rearrange("p (k j f) -> p k j f", k=3, j=3)
    for k in range(3):
        dst = Rv if k == 0 else Tkv
        nc.vector.tensor_tensor(
            dst, A[:, 0:3, k].unsqueeze(2).broadcast_to([128, 3, 3, SFD]),
            Wv4[:, k].unsqueeze(1).broadcast_to([128, 3, 3, SFD]), OP.mult)
        if k:
            nc.vector.tensor_tensor(R[:], R[:], Tk[:], OP.add)

    # v_j = (sum_i sp_i R_ij - sg_j)/SEC
    v = pers.tile([128, 3 * SFD], BF16, tag="v", name="v")
    vv = v[:].rearrange("p (j f) -> p j f", j=3)
    Pv = pol.tile([128, 9 * SFD], BF16, tag="Pv", name="Pv")
    Pvv = Pv[:].rearrange("p (i j f) -> p i j f", i=3, j=3)
    nc.vector.tensor_tensor(
        Pvv, sp3.unsqueeze(2).broadcast_to([128, 3, 3, SFD]), Rv, OP.mult)
    nc.vector.tensor_tensor(vv, Pvv[:, 0], Pvv[:, 1], OP.add)
    nc.vector.tensor_tensor(vv, vv, Pvv[:, 2], OP.add)
    nc.vector.tensor_tensor(vv, vv, sg3, OP.subtract)
    nc.vector.tensor_scalar(v[:], v[:], 1.0 / SEC, None, OP.mult)

    pol.release()
    # ---------------- phase 3: rotation residual ----------------
    wk3 = tc.alloc_tile_pool(name="wk3", bufs=2)
    work = wk3
    for c in range(NCH_KP):
        p3 = pb[c][:].rearrange("p (d k s) -> p d k s", d=3, k=SEC)
        g3 = gb[c][:].rearrange("p (d k s) -> p d k s", d=3, k=SEC)
        Rc = Rv[:, :, :, c * S:(c + 1) * S]              # [128,3,3,S]
        vc = vv[:, :, c * S:(c + 1) * S]                 # [128,3,S]
        T0 = work.tile([128, 1200], BF16, tag="r0", name="r0")
        T1_ = work.tile([128, 1200], BF16, tag="r1", name="r1")
        T2_ = work.tile([128, 1200], BF16, tag="r2", name="r2")
        for i, Td in enumerate((T0, T1_, T2_)):
            nc.vector.tensor_tensor(
                Td[:].rearrange("p (j k s) -> p j k s", j=3, k=SEC),
                p3[:, i].unsqueeze(1).broadcast_to([128, 3, SEC, S]),
                Rc[:, i].unsqueeze(2).broadcast_to([128, 3, SEC, S]), OP.mult)
        nc.vector.tensor_tensor(T0[:], T0[:], T1_[:], OP.add)
        Bt = work.tile([128, 1200], BF16, tag="rB", name="rB")
        nc.vector.tensor_tensor(
            Bt[:].rearrange("p (j k s) -> p j k s", j=3, k=SEC),
            g3, vc.unsqueeze(2).broadcast_to([128, 3, SEC, S]), OP.add)
        nc.vector.tensor_tensor(Bt[:], Bt[:], T2_[:], OP.subtract)
        dmp = scr.tile([128, 1200], BF16, tag="dump1200", name="dump1200")
        nc.vector._custom_dve(SL1_DIFF, out=dmp[:], in0=T0[:], in1=Bt[:],
                              s0=-1.0, s1=1.0, imm2=0.5,
                              accum_out=acc[:, C_ROT + c:C_ROT + c + 1])

    wk3.release()
    nc.sync.dma_start(out[:], acc[:])


_CACHE = {}


def _build():
    if "nc" in _CACHE:
        return _CACHE["nc"]
    nc = bacc.Bacc("TRN2", target_bir_lowering=False, debug=False,
                   enable_asserts=False, num_devices=N_CORES)
    aps = {
        "pk": nc.dram_tensor("pk", [NCH_KP, 128, 1200], BF16, kind="ExternalInput").ap(),
        "gk": nc.dram_tensor("gk", [NCH_KP, 128, 1200], BF16, kind="ExternalInput").ap(),
        "lgn": nc.dram_tensor("lgn", [NCH_CE, 128, NS * T_CE], BF16, kind="ExternalInput").ap(),
        "lgf": nc.dram_tensor("lgf", [NCH_CE, 100, FFL], BF16, kind="ExternalInput").ap(),
        "lb": nc.dram_tensor("lb", [NCH_CE, 128, T_CE], BF16, kind="ExternalInput").ap(),
        "ob": nc.dram_tensor("ob", [100, 5], BF16, kind="ExternalInput").ap(),
        "out": nc.dram_tensor("out", [128, NACC], FP32, kind="ExternalOutput").ap(),
    }
    with tile.TileContext(nc) as tc:
        with ExitStack() as ctx:
            with nc.allow_low_precision(reason="bf16 5-term tree reduces; validated"):
                _emit(ctx, tc, aps)
    nc.compile()
    _CACHE["nc"] = nc
    return nc


def _shard_inputs(pred_keypoints, gt_keypoints, pred_section_logits, gt_section_label):
    bf = ml_dtypes.bfloat16
    pkh = np.asarray(pred_keypoints, dtype=np.float32).reshape(
        N_CORES, NCH_KP, 128, S, SEC, 3).transpose(0, 1, 2, 5, 4, 3)
    pkh = np.ascontiguousarray(pkh).reshape(N_CORES, NCH_KP, 128, 1200).astype(bf)
    gkh = np.asarray(gt_keypoints, dtype=np.float32).reshape(
        N_CORES, NCH_KP, 128, S, SEC, 3).transpose(0, 1, 2, 5, 4, 3)
    gkh = np.ascontiguousarray(gkh).reshape(N_CORES, NCH_KP, 128, 1200).astype(bf)
    lg32 = np.asarray(pred_section_logits, dtype=np.float32)
    lgnh = lg32.reshape(N_CORES, NCH_CE, 128, T_CE, NS).transpose(0, 1, 2, 4, 3)
    lgnh = np.ascontiguousarray(lgnh).reshape(N_CORES, NCH_CE, 128, NS * T_CE).astype(bf)
    lgfh = lg32.reshape(N_CORES, NCH_CE, FFL, 100).transpose(0, 1, 3, 2)
    lgfh = np.ascontiguousarray(lgfh).astype(bf)
    lbh = np.asarray(gt_section_label).reshape(N_CORES, NCH_CE, 128, T_CE).astype(bf)
    ob = np.zeros((100, 5), dtype=np.float32)
    for g in range(5):
        ob[g * 20:(g + 1) * 20, g] = 1.0
    ob = ob.astype(bf)
    return [{"pk": pkh[i], "gk": gkh[i], "lgn": lgnh[i], "lgf": lgfh[i],
             "lb": lbh[i], "ob": ob} for i in range(N_CORES)]


def combine_accs(accs):
    tot = np.zeros(NACC, dtype=np.float64)
    lse = 0.0
    for a in accs:
        a64 = a.astype(np.float64)
        tot += a64.sum(axis=0)
        for g in range(5):
            col = a64[:, C_LSE + g]
            for q in range(4):
                lse += col[32 * q:32 * q + 5].sum()
    ly = tot[C_LY:C_LY + NCH_CE].sum()
    kp = tot[C_KP:C_KP + NCH_KP].sum()
    rot = tot[C_ROT:C_ROT + NCH_KP].sum()
    cent = tot[C_CENT]
    total = (1.0 * (lse - ly) / (B * K)
             + 4.0 * kp / (B * K * 3)
             + 5.0 * rot / (B * K * 3)
             + 6.0 * cent / (B * S * 3))
    return np.float32(total)


def kernel(**inputs) -> np.ndarray:
    nc = _build()
    in_maps = _shard_inputs(**inputs)
    res = run_bass_kernel_spmd(nc, in_maps, list(range(N_CORES))).results
    return combine_accs([res[i]["out"] for i in range(N_CORES)])


# revision 19
# speedup vs baseline: 2.9279x; 1.0013x over previous
"""Trainium2 Bass kernel for nn_KPLoss_377957122199 (v2, engine-rebalanced).

loss = 1*CE + 4*smoothL1(kp) + 5*smoothL1(Procrustes rot residual)
     + 6*smoothL1(section-center diff)

Data-parallel over 8 cores (batch 8192 -> 1024/core). Key design:
  * custom DVE ops: SL1_DIFF (fused smooth-L1 sum of (in0-in1) in one
    vector op via sum f(d) = sum (d - 0.5*clamp(d))*clamp(d)), and
    ONEHOT_DOT (sum_t logits[y_t, t] via PageIdx compare, one op/chunk)
  * CE sum-of-exp on the idle TensorEngine: logits shipped twice
    (n-major [128,(n,t)] for ONEHOT; flat-transposed [100,4096] so a
    block-ones matmul reduces NS=20 on partitions into PSUM), ln reads
    PSUM packed 4 chunks/ACT via matmul tile_position.
  * keypoints host-deinterleaved to [d,k,s] bf16; kept in SBUF across
    both passes; H products/reductions and the rotation residual use
    stride-0 broadcast views (no materialized broadcasts).
  * batched 3x3 polar: 9 components contiguous [128,9*160] bf16,
    cofactors via shifted views of a 6x6-duplicated tile, scaled-Newton
    x3 + one Newton-Schulz polish, guarded (|det| clamp + Sign).
  * single ACT table set (exp/ln/sign only) - one table load.
"""

import sys
for _p in ("/opt/trn_rl_repo", "/root/.axon_site/_ro/trn_rl_repo"):
    if _p not in sys.path:
        sys.path.insert(0, _p)

import os
from contextlib import ExitStack
from operator import add as _add_op

import numpy as np
import ml_dtypes

import concourse.bass as bass
import concourse.bacc as bacc
import concourse.mybir as mybir
import concourse.tile as tile
from concourse.bass_utils import run_bass_kernel_spmd

# ---- custom DVE ops (registered at import) --------------------------------
import concourse.dve_ops as dve_ops
from concourse.dve_ops import DveOp, OPS
from concourse.dve_spec import (
    C0, C1, C2, PageIdx, Spec, Src0, Src1, Zero,
    _has_src1, eq, lower, maxx, minn, select,
)
from concourse.dve_uop import DveOpSpec


def _sl1_ref(in0, in1, s0, s1, imm2):
    d = in0.astype(np.float32) - in1.astype(np.float32)
    t = np.clip(d, s0, s1)
    return (d - imm2 * t) * t


def _oh_ref(in0, in1, s0, s1, imm2):
    raise NotImplementedError


def _register(name, spec, subdim):
    if name in dve_ops._SUB_OPCODE_FOR_NAME:
        return next(o for o in OPS if o.name == name)
    row = dve_ops._CUSTOM_DVE_ROW_BASE + len(OPS)
    assert row < 0x20
    op = DveOp(name, spec, subdim=subdim, uops_sha={})
    for ver in ("v3", "v4"):
        s = DveOpSpec(name=name, opcode=row, uops=lower(spec, ver=ver),
                      rd1_en=_has_src1(spec))
        op.uops_sha[ver] = s.sha(ver)
    OPS.append(op)
    dve_ops._SUB_OPCODE_FOR_NAME[name] = row
    return op


_d = Src0 - Src1
_t = minn(maxx(_d, C0), C1)
SL1_DIFF = _register("SL1_DIFF", Spec(body=(_d - _t * C2) * _t, accum=_add_op,
                                      reference=_sl1_ref), subdim=False)
_pg = PageIdx(C0, C1)
ONEHOT_DOT = _register("ONEHOT_DOT",
                       Spec(body=select(eq(Src1, _pg), Src0, Zero),
                            accum=_add_op, reference=_oh_ref), subdim=True)

FP32 = mybir.dt.float32
BF16 = mybir.dt.bfloat16
AX = mybir.AxisListType
OP = mybir.AluOpType
AF = mybir.ActivationFunctionType

N_CORES = 8
B, K, NS, SEC = 8192, 400, 20, 20
S = K // SEC                    # 20 sections / sample
BC = B // N_CORES               # 1024 samples / core
NCH_KP = BC // 128              # 8 keypoint chunks
SFD = NCH_KP * S                # 160 sections per partition
NCH_CE = 20                     # CE chunks
TOKC = BC * K // NCH_CE         # 20480 tokens / CE chunk
T_CE = TOKC // 128              # 160 tokens / partition (n-major layout)
FFL = TOKC * NS // 100          # 4096 cols in flat [100, .] layout

N_ITER = 2                      # polar Newton iterations

# acc column map
C_LSE = 0                       # 5 cols (groups of 4 chunks; rows 32q+0..4)
C_LY = C_LSE + 5                # 20 cols
C_KP = C_LY + NCH_CE            # 8
C_ROT = C_KP + NCH_KP           # 8
C_CENT = C_ROT + NCH_KP         # 1
NACC = C_CENT + 1


def _emit(ctx, tc, aps):
    nc = tc.nc
    pk, gk, lgn, lgf, lb, ob, out = (aps[k] for k in
                                     ("pk", "gk", "lgn", "lgf", "lb", "ob", "out"))

    pers = ctx.enter_context(tc.tile_pool(name="pers", bufs=1))
    scr = ctx.enter_context(tc.tile_pool(name="scr", bufs=1))
    cep = ctx.enter_context(tc.tile_pool(name="ce", bufs=2))
    psp = ctx.enter_context(tc.tile_pool(name="ps", bufs=1, space="PSUM"))

    acc = pers.tile([128, NACC], FP32, tag="acc", name="acc")
    oneblk = pers.tile([100, 5], BF16, tag="oneblk", name="oneblk")
    nc.sync.dma_start(oneblk[:], ob)
    lnhalf = pers.tile([128, 1], FP32, tag="lnhalf", name="lnhalf")
    nc.gpsimd.memset(lnhalf[:], float(np.log(0.5)))

    # keypoint chunks persist across phase 1 and 3
    pb = [pers.tile([128, 1200], BF16, tag=f"pb{c}", name=f"pb{c}") for c in range(NCH_KP)]
    gb = [pers.tile([128, 1200], BF16, tag=f"gb{c}", name=f"gb{c}") for c in range(NCH_KP)]
    for c in range(NCH_KP):
        nc.sync.dma_start(pb[c][:], pk[c])
        nc.sync.dma_start(gb[c][:], gk[c])

    # ---------------- cross entropy ----------------
    psum = psp.tile([128, FFL], FP32, tag="mm", name="mm")

    def ce_chunk(c):
        lgnc = cep.tile([128, NS * T_CE], BF16, tag="lgn", name="lgn")
        nc.sync.dma_start(lgnc[:], lgn[c])
        lbc = cep.tile([128, T_CE], BF16, tag="lbc", name="lbc")
        nc.sync.dma_start(lbc[:], lb[c])
        lgfc = cep.tile([100, FFL], BF16, tag="lgf", name="lgf")
        nc.sync.dma_start(lgfc[:], lgf[c])

        # l_y: one custom op
        dmp = scr.tile([128, NS * T_CE], BF16, tag="dmp", name="dmp")
        nc.vector._custom_dve(
            ONEHOT_DOT,
            out=dmp[:].rearrange("p (n t) -> p n t", n=NS),
            in0=lgnc[:].rearrange("p (n t) -> p n t", n=NS),
            in1=lbc[:].unsqueeze(1).broadcast_to([128, NS, T_CE]),
            s0=0.0, s1=1.0, accum_out=acc[:, C_LY + c:C_LY + c + 1])

        # lse: exp (scalar) -> block-ones matmul (PE) -> ln on packed PSUM
        ex = lgfc
        nc.scalar.activation(ex[:], lgfc[:], AF.Exp)
        q = c % 4
        for h in range(FFL // 512):
            nc.tensor.matmul(
                psum[32 * q:32 * q + 5, h * 512:(h + 1) * 512],
                oneblk[:], ex[:, h * 512:(h + 1) * 512],
                start=True, stop=True, tile_position=(0, 32 * q))
        if q == 3:
            g = c // 4
            lnd = scr.tile([101, FFL], BF16, tag="lnd", name="lnd")
            nc.scalar.activation(lnd[:], psum[0:101, :], AF.Ln,
                                 accum_out=acc[0:101, C_LSE + g:C_LSE + g + 1])

    for c in range(12):
        ce_chunk(c)

    # ---------------- phase 1: keypoints ----------------
    H = pers.tile([128, 9 * SFD], BF16, tag="H", name="H")
    sp = pers.tile([128, 3 * SFD], BF16, tag="sp", name="sp")
    sg = pers.tile([128, 3 * SFD], BF16, tag="sg", name="sg")

    wk1 = tc.alloc_tile_pool(name="wk1", bufs=2)
    work = wk1
    for c in range(NCH_KP):
        p3 = pb[c][:].rearrange("p (d f) -> p d f", d=3)        # [128,3,400]
        g3 = gb[c][:].rearrange("p (d f) -> p d f", d=3)
        dmp = scr.tile([128, 1200], BF16, tag="dump1200", name="dump1200")
        nc.vector._custom_dve(SL1_DIFF, out=dmp[:], in0=pb[c][:], in1=gb[c][:],
                              s0=-1.0, s1=1.0, imm2=0.5,
                              accum_out=acc[:, C_KP + c:C_KP + c + 1])
        # H products: T[m=(i,j)] = g_i * p_j over (k,s)
        T = work.tile([128, 9 * 400], BF16, tag="hT", name="hT")
        nc.vector.tensor_tensor(
            T[:].rearrange("p (i j f) -> p i j f", i=3, j=3),
            g3.unsqueeze(2).broadcast_to([128, 3, 3, 400]),
            p3.unsqueeze(1).broadcast_to([128, 3, 3, 400]), OP.mult)
        # k-tree: 20 -> 10 -> 5 -> reduce
        T4 = T[:].rearrange("p (m k s) -> p m k s", m=9, k=SEC)
        A1 = work.tile([128, 9 * 10 * S], BF16, tag="hA1", name="hA1")
        A1v = A1[:].rearrange("p (m k s) -> p m k s", m=9, k=10, s=S)
        nc.vector.tensor_tensor(A1v, T4[:, :, 0:10], T4[:, :, 10:20], OP.add)
        A2 = work.tile([128, 9 * 5 * S], BF16, tag="hA2", name="hA2")
        A2v = A2[:].rearrange("p (m k s) -> p m k s", m=9, k=5, s=S)
        nc.vector.tensor_tensor(A2v, A1v[:, :, 0:5], A1v[:, :, 5:10], OP.add)
        nc.vector.tensor_reduce(
            H[:].rearrange("p (m f) -> p m f", m=9)[:, :, c * S:(c + 1) * S],
            A2[:].rearrange("p (m k s) -> p m s k", m=9, k=5, s=S),
            axis=AX.X, op=OP.add)
        # point sums over k (single strided reduce per tensor)
        for src, dst in ((p3, sp), (g3, sg)):
            nc.vector.tensor_reduce(
                dst[:].rearrange("p (d f) -> p d f", d=3)[:, :, c * S:(c + 1) * S],
                src.rearrange("p d (k s) -> p d s k", k=SEC),
                axis=AX.X, op=OP.add)

    wk1.release()
    # center loss: smoothL1((sp-sg)/SEC) over [128, 3*SFD]
    sps = pers.tile([128, 3 * SFD], BF16, tag="sps", name="sps")
    sgs = pers.tile([128, 3 * SFD], BF16, tag="sgs", name="sgs")
    nc.vector.tensor_scalar(sps[:], sp[:], 1.0 / SEC, None, OP.mult)
    nc.vector.tensor_scalar(sgs[:], sg[:], 1.0 / SEC, None, OP.mult)
    dmpc = scr.tile([128, 3 * SFD], BF16, tag="dmpc", name="dmpc")
    nc.vector._custom_dve(SL1_DIFF, out=dmpc[:], in0=sps[:], in1=sgs[:],
                          s0=-1.0, s1=1.0, imm2=0.5,
                          accum_out=acc[:, C_CENT:C_CENT + 1])

    # H -= sg_i * sp_j / SEC
    sp3 = sp[:].rearrange("p (d f) -> p d f", d=3)
    sg3 = sg[:].rearrange("p (d f) -> p d f", d=3)
    M = scr.tile([128, 9 * SFD], BF16, tag="hcM", name="hcM")
    nc.vector.tensor_tensor(
        M[:].rearrange("p (i j f) -> p i j f", i=3, j=3),
        sg3.unsqueeze(2).broadcast_to([128, 3, 3, SFD]),
        sp3.unsqueeze(1).broadcast_to([128, 3, 3, SFD]), OP.mult)
    nc.vector.tensor_scalar(M[:], M[:], 1.0 / SEC, None, OP.mult)
    nc.vector.tensor_tensor(H[:], H[:], M[:], OP.subtract)

    # ---------------- polar decomposition (batched 3x3, bf16) ----------------
    pol = tc.alloc_tile_pool(name="pol", bufs=1)
    A66 = pol.tile([128, 36 * SFD], BF16, tag="A66", name="A66")
    A = A66[:].rearrange("p (a b f) -> p a b f", a=6, b=6)
    X = A[:, 0:3, 0:3]                                  # X lives inside A66
    H4 = H[:].rearrange("p (i j f) -> p i j f", i=3, j=3)
    nc.vector.tensor_copy(X, H4)
    Cf = pol.tile([128, 9 * SFD], BF16, tag="cof", name="cof")
    C3v = Cf[:].rearrange("p (i j f) -> p i j f", i=3, j=3)
    SX = pol.tile([128, 9 * SFD], BF16, tag="sqX", name="sqX")
    det = pol.tile([128, SFD], BF16, tag="det", name="det")
    t160a = pol.tile([128, SFD], FP32, tag="t160a", name="t160a")
    t160b = pol.tile([128, SFD], FP32, tag="t160b", name="t160b")
    adet = pol.tile([128, SFD], FP32, tag="adet", name="adet")
    sgn = pol.tile([128, SFD], BF16, tag="sgn", name="sgn")
    nx2 = pol.tile([128, SFD], BF16, tag="nx2", name="nx2")
    nc2_ = pol.tile([128, SFD], BF16, tag="nc2", name="nc2")
    zln = pol.tile([128, SFD], FP32, tag="zln", name="zln")
    hz = pol.tile([128, SFD], BF16, tag="hz", name="hz")
    wz = pol.tile([128, SFD], BF16, tag="wz", name="wz")
    wf = pol.tile([128, SFD], FP32, tag="wf", name="wf")

    def frob(dst, src4):
        sxw = SX[:].rearrange("p (i j f) -> p i j f", i=3, j=3)
        nc.vector.tensor_tensor(sxw, src4, src4, OP.mult)
        sx = SX[:].rearrange("p (m f) -> p m f", m=9)
        q1 = pol.tile([128, 4 * SFD], BF16, tag="fq1", name="fq1")
        q1v = q1[:].rearrange("p (m f) -> p m f", m=4)
        nc.vector.tensor_tensor(q1v, sx[:, 0:4], sx[:, 4:8], OP.add)
        q2 = pol.tile([128, 2 * SFD], BF16, tag="fq2", name="fq2")
        q2v = q2[:].rearrange("p (m f) -> p m f", m=2)
        nc.vector.tensor_tensor(q2v, q1v[:, 0:2], q1v[:, 2:4], OP.add)
        nc.vector.tensor_tensor(dst.unsqueeze(1), q2v[:, 0:1], q2v[:, 1:2], OP.add)
        nc.vector.tensor_tensor(dst, dst, sx[:, 8], OP.add)

    for it in range(N_ITER):
        # duplicate X -> A66 quadrants
        nc.vector.tensor_copy(A[:, 0:3, 3:6], X)
        nc.vector.tensor_copy(A[:, 3:6, :], A[:, 0:3, :])
        # cofactors: C[i][j] = A[i+1][j+1]A[i+2][j+2] - A[i+1][j+2]A[i+2][j+1]
        T1 = pol.tile([128, 9 * SFD], BF16, tag="ct1", name="ct1")
        nc.vector.tensor_tensor(
            T1[:].rearrange("p (i j f) -> p i j f", i=3, j=3),
            A[:, 1:4, 1:4], A[:, 2:5, 2:5], OP.mult)
        T2 = pol.tile([128, 9 * SFD], BF16, tag="ct2", name="ct2")
        nc.vector.tensor_tensor(
            T2[:].rearrange("p (i j f) -> p i j f", i=3, j=3),
            A[:, 1:4, 2:5], A[:, 2:5, 1:4], OP.mult)
        nc.vector.tensor_tensor(Cf[:], T1[:], T2[:], OP.subtract)
        # det = sum_j X[0][j] * C[0][j]
        P0 = pol.tile([128, 3 * SFD], BF16, tag="dp0", name="dp0")
        P0v = P0[:].rearrange("p (j f) -> p j f", j=3)
        nc.vector.tensor_tensor(P0v, X[:, 0], C3v[:, 0], OP.mult)
        nc.vector.tensor_tensor(det[:].unsqueeze(1), P0v[:, 0:1], P0v[:, 1:2], OP.add)
        nc.vector.tensor_tensor(det[:], det[:], P0v[:, 2], OP.add)
        # guards + zeta = exp(0.25 ln(nC2/nX2) - 0.5 ln|det|)
        frob(nx2[:], X)
        frob(nc2_[:], C3v)
        nc.vector.tensor_scalar(nx2[:], nx2[:], 1e-12, None, OP.max)
        nc.vector.tensor_scalar(nc2_[:], nc2_[:], 1e-12, None, OP.max)
        nc.scalar.activation(adet[:], det[:], AF.Abs)
        nc.vector.tensor_scalar(adet[:], adet[:], 1e-6, None, OP.max)
        nc.vector.tensor_scalar(sgn[:], det[:], 0.0, None, OP.is_ge)
        nc.vector.tensor_scalar(sgn[:], sgn[:], 2.0, -1.0, OP.mult, OP.add)
        nc.scalar.activation(t160a[:], nc2_[:], AF.Ln)
        nc.scalar.activation(t160b[:], nx2[:], AF.Ln)
        nc.vector.tensor_tensor(t160a[:], t160a[:], t160b[:], OP.subtract)
        nc.scalar.activation(t160b[:], adet[:], AF.Ln)
        nc.vector.tensor_scalar(t160b[:], t160b[:], -2.0, None, OP.mult)
        nc.vector.tensor_tensor(zln[:], t160a[:], t160b[:], OP.add)
        nc.vector.tensor_scalar(zln[:], zln[:], 0.25, None, OP.mult)
        # hz = 0.5*zeta = exp(zln + ln 0.5); w = 0.5*sgn/(zeta*|det|)
        nc.scalar.activation(hz[:], zln[:], AF.Exp, bias=lnhalf[:])
        nc.scalar.activation(t160a[:], zln[:], AF.Exp)
        nc.vector.tensor_tensor(t160b[:], t160a[:], adet[:], OP.mult)
        nc.vector.tensor_scalar(t160b[:], t160b[:], 2.0, None, OP.mult)
        nc.vector.reciprocal_approx_fast(wf[:], t160b[:])
        nc.vector.tensor_tensor(wz[:], wf[:], sgn[:], OP.mult)
        # X = X*hz + C*w  (broadcast over 9 components)
        hzb = hz[:].unsqueeze(1).unsqueeze(1).broadcast_to([128, 3, 3, SFD])
        wzb = wz[:].unsqueeze(1).unsqueeze(1).broadcast_to([128, 3, 3, SFD])
        U1 = pol.tile([128, 9 * SFD], BF16, tag="u1", name="u1")
        U1v = U1[:].rearrange("p (i j f) -> p i j f", i=3, j=3)
        nc.vector.tensor_tensor(U1v, X, hzb, OP.mult)
        U2 = pol.tile([128, 9 * SFD], BF16, tag="u2", name="u2")
        U2v = U2[:].rearrange("p (i j f) -> p i j f", i=3, j=3)
        nc.vector.tensor_tensor(U2v, C3v, wzb, OP.mult)
        nc.vector.tensor_tensor(X, U1v, U2v, OP.add)

    # Newton-Schulz polish: R = X (1.5 I - 0.5 X^T X)
    Y = pol.tile([128, 9 * SFD], BF16, tag="Y", name="Y")
    Yv = Y[:].rearrange("p (i j f) -> p i j f", i=3, j=3)
    Tk = pol.tile([128, 9 * SFD], BF16, tag="Tk", name="Tk")
    Tkv = Tk[:].rearrange("p (i j f) -> p i j f", i=3, j=3)
    for k in range(3):
        xk = A[:, k, 0:3]                               # [128, 3, SFD] = X[k][*]
        dst = Yv if k == 0 else Tkv
        nc.vector.tensor_tensor(
            dst, xk.unsqueeze(2).broadcast_to([128, 3, 3, SFD]),
            xk.unsqueeze(1).broadcast_to([128, 3, 3, SFD]), OP.mult)
        if k:
            nc.vector.tensor_tensor(Y[:], Y[:], Tk[:], OP.add)
    W = pol.tile([128, 9 * SFD], BF16, tag="W", name="W")
    nc.vector.tensor_scalar(W[:], Y[:], -0.5, None, OP.mult)
    Wv = W[:].rearrange("p (m f) -> p m f", m=9)
    for m in (0, 4, 8):
        nc.vector.tensor_scalar(Wv[:, m], Wv[:, m], 1.5, None, OP.add)
    R = pers.tile([128, 9 * SFD], BF16, tag="R", name="R")
    Rv = R[:].rearrange("p (i j f) -> p i j f", i=3, j=3)
    Wv4 = W[:].rearrange("p (k j f) -> p k j f", k=3, j=3)
    for k in range(3):
        dst = Rv if k == 0 else Tkv
        nc.vector.tensor_tensor(
            dst, A[:, 0:3, k].unsqueeze(2).broadcast_to([128, 3, 3, SFD]),
            Wv4[:, k].unsqueeze(1).broadcast_to([128, 3, 3, SFD]), OP.mult)
        if k:
            nc.vector.tensor_tensor(R[:], R[:], Tk[:], OP.add)

    # v_j = (sum_i sp_i R_ij - sg_j)/SEC
    v = pers.tile([128, 3 * SFD], BF16, tag="v", name="v")
    vv = v[:].rearrange("p (j f) -> p j f", j=3)
    Pv = pol.tile([128, 9 * SFD], BF16, tag="Pv", name="Pv")
    Pvv = Pv[:].rearrange("p (i j f) -> p i j f", i=3, j=3)
    nc.vector.tensor_tensor(
        Pvv, sp3.unsqueeze(2).broadcast_to([128, 3, 3, SFD]), Rv, OP.mult)
    nc.vector.tensor_tensor(vv, Pvv[:, 0], Pvv[:, 1], OP.add)
    nc.vector.tensor_tensor(vv, vv, Pvv[:, 2], OP.add)
    nc.vector.tensor_tensor(vv, vv, sg3, OP.subtract)
    nc.vector.tensor_scalar(v[:], v[:], 1.0 / SEC, None, OP.mult)

    pol.release()
    # ---------------- phase 3: rotation residual ----------------
    wk3 = tc.alloc_tile_pool(name="wk3", bufs=2)
    work = wk3
    for c in range(NCH_KP):
        p3 = pb[c][:].rearrange("p (d k s) -> p d k s", d=3, k=SEC)
        g3 = gb[c][:].rearrange("p (d k s) -> p d k s", d=3, k=SEC)
        Rc = Rv[:, :, :, c * S:(c + 1) * S]              # [128,3,3,S]
        vc = vv[:, :, c * S:(c + 1) * S]                 # [128,3,S]
        T0 = work.tile([128, 1200], BF16, tag="r0", name="r0")
        T1_ = work.tile([128, 1200], BF16, tag="r1", name="r1")
        T2_ = work.tile([128, 1200], BF16, tag="r2", name="r2")
        for i, Td in enumerate((T0, T1_, T2_)):
            nc.vector.tensor_tensor(
                Td[:].rearrange("p (j k s) -> p j k s", j=3, k=SEC),
                p3[:, i].unsqueeze(1).broadcast_to([128, 3, SEC, S]),
                Rc[:, i].unsqueeze(2).broadcast_to([128, 3, SEC, S]), OP.mult)
        nc.vector.tensor_tensor(T0[:], T0[:], T1_[:], OP.add)
        Bt = work.tile([128, 1200], BF16, tag="rB", name="rB")
        nc.gpsimd.tensor_tensor(
            Bt[:].rearrange("p (j k s) -> p j k s", j=3, k=SEC),
            g3, vc.unsqueeze(2).broadcast_to([128, 3, SEC, S]), OP.add)
        nc.vector.tensor_tensor(Bt[:], Bt[:], T2_[:], OP.subtract)
        dmp = scr.tile([128, 1200], BF16, tag="dump1200", name="dump1200")
        nc.vector._custom_dve(SL1_DIFF, out=dmp[:], in0=T0[:], in1=Bt[:],
                              s0=-1.0, s1=1.0, imm2=0.5,
                              accum_out=acc[:, C_ROT + c:C_ROT + c + 1])

    wk3.release()
    for c in range(12, NCH_CE):
        ce_chunk(c)
    nc.sync.dma_start(out[:], acc[:])


_CACHE = {}


def _build():
    if "nc" in _CACHE:
        return _CACHE["nc"]
    nc = bacc.Bacc("TRN2", target_bir_lowering=False, debug=False,
                   enable_asserts=False, num_devices=N_CORES)
    aps = {
        "pk": nc.dram_tensor("pk", [NCH_KP, 128, 1200], BF16, kind="ExternalInput").ap(),
        "gk": nc.dram_tensor("gk", [NCH_KP, 128, 1200], BF16, kind="ExternalInput").ap(),
        "lgn": nc.dram_tensor("lgn", [NCH_CE, 128, NS * T_CE], BF16, kind="ExternalInput").ap(),
        "lgf": nc.dram_tensor("lgf", [NCH_CE, 100, FFL], BF16, kind="ExternalInput").ap(),
        "lb": nc.dram_tensor("lb", [NCH_CE, 128, T_CE], BF16, kind="ExternalInput").ap(),
        "ob": nc.dram_tensor("ob", [100, 5], BF16, kind="ExternalInput").ap(),
        "out": nc.dram_tensor("out", [128, NACC], FP32, kind="ExternalOutput").ap(),
    }
    with tile.TileContext(nc) as tc:
        with ExitStack() as ctx:
            with nc.allow_low_precision(reason="bf16 5-term tree reduces; validated"):
                _emit(ctx, tc, aps)
    nc.compile()
    _CACHE["nc"] = nc
    return nc


def _shard_inputs(pred_keypoints, gt_keypoints, pred_section_logits, gt_section_label):
    bf = ml_dtypes.bfloat16
    pkh = np.asarray(pred_keypoints, dtype=np.float32).reshape(
        N_CORES, NCH_KP, 128, S, SEC, 3).transpose(0, 1, 2, 5, 4, 3)
    pkh = np.ascontiguousarray(pkh).reshape(N_CORES, NCH_KP, 128, 1200).astype(bf)
    gkh = np.asarray(gt_keypoints, dtype=np.float32).reshape(
        N_CORES, NCH_KP, 128, S, SEC, 3).transpose(0, 1, 2, 5, 4, 3)
    gkh = np.ascontiguousarray(gkh).reshape(N_CORES, NCH_KP, 128, 1200).astype(bf)
    lg32 = np.asarray(pred_section_logits, dtype=np.float32)
    lgnh = lg32.reshape(N_CORES, NCH_CE, 128, T_CE, NS).transpose(0, 1, 2, 4, 3)
    lgnh = np.ascontiguousarray(lgnh).reshape(N_CORES, NCH_CE, 128, NS * T_CE).astype(bf)
    lgfh = lg32.reshape(N_CORES, NCH_CE, FFL, 100).transpose(0, 1, 3, 2)
    lgfh = np.ascontiguousarray(lgfh).astype(bf)
    lbh = np.asarray(gt_section_label).reshape(N_CORES, NCH_CE, 128, T_CE).astype(bf)
    ob = np.zeros((100, 5), dtype=np.float32)
    for g in range(5):
        ob[g * 20:(g + 1) * 20, g] = 1.0
    ob = ob.astype(bf)
    return [{"pk": pkh[i], "gk": gkh[i], "lgn": lgnh[i], "lgf": lgfh[i],
             "lb": lbh[i], "ob": ob} for i in range(N_CORES)]


def combine_accs(accs):
    tot = np.zeros(NACC, dtype=np.float64)
    lse = 0.0
    for a in accs:
        a64 = a.astype(np.float64)
        tot += a64.sum(axis=0)
        for g in range(5):
            col = a64[:, C_LSE + g]
            for q in range(4):
                lse += col[32 * q:32 * q + 5].sum()
    ly = tot[C_LY:C_LY + NCH_CE].sum()
    kp = tot[C_KP:C_KP + NCH_KP].sum()
    rot = tot[C_ROT:C_ROT + NCH_KP].sum()
    cent = tot[C_CENT]
    total = (1.0 * (lse - ly) / (B * K)
             + 4.0 * kp / (B * K * 3)
             + 5.0 * rot / (B * K * 3)
             + 6.0 * cent / (B * S * 3))
    return np.float32(total)


def kernel(**inputs) -> np.ndarray:
    nc = _build()
    in_maps = _shard_inputs(**inputs)
    res = run_bass_kernel_spmd(nc, in_maps, list(range(N_CORES))).results
    return combine_accs([res[i]["out"] for i in range(N_CORES)])


# revision 20
# speedup vs baseline: 2.9364x; 1.0029x over previous
"""Trainium2 Bass kernel for nn_KPLoss_377957122199 (v2, engine-rebalanced).

loss = 1*CE + 4*smoothL1(kp) + 5*smoothL1(Procrustes rot residual)
     + 6*smoothL1(section-center diff)

Data-parallel over 8 cores (batch 8192 -> 1024/core). Key design:
  * custom DVE ops: SL1_DIFF (fused smooth-L1 sum of (in0-in1) in one
    vector op via sum f(d) = sum (d - 0.5*clamp(d))*clamp(d)), and
    ONEHOT_DOT (sum_t logits[y_t, t] via PageIdx compare, one op/chunk)
  * CE sum-of-exp on the idle TensorEngine: logits shipped twice
    (n-major [128,(n,t)] for ONEHOT; flat-transposed [100,4096] so a
    block-ones matmul reduces NS=20 on partitions into PSUM), ln reads
    PSUM packed 4 chunks/ACT via matmul tile_position.
  * keypoints host-deinterleaved to [d,k,s] bf16; kept in SBUF across
    both passes; H products/reductions and the rotation residual use
    stride-0 broadcast views (no materialized broadcasts).
  * batched 3x3 polar: 9 components contiguous [128,9*160] bf16,
    cofactors via shifted views of a 6x6-duplicated tile, scaled-Newton
    x3 + one Newton-Schulz polish, guarded (|det| clamp + Sign).
  * single ACT table set (exp/ln/sign only) - one table load.
"""

import sys
for _p in ("/opt/trn_rl_repo", "/root/.axon_site/_ro/trn_rl_repo"):
    if _p not in sys.path:
        sys.path.insert(0, _p)

import os
from contextlib import ExitStack
from operator import add as _add_op

import numpy as np
import ml_dtypes

import concourse.bass as bass
import concourse.bacc as bacc
import concourse.mybir as mybir
import concourse.tile as tile
from concourse.bass_utils import run_bass_kernel_spmd

# ---- custom DVE ops (registered at import) --------------------------------
import concourse.dve_ops as dve_ops
from concourse.dve_ops import DveOp, OPS
from concourse.dve_spec import (
    C0, C1, C2, PageIdx, Spec, Src0, Src1, Zero,
    _has_src1, eq, lower, maxx, minn, select,
)
from concourse.dve_uop import DveOpSpec


def _sl1_ref(in0, in1, s0, s1, imm2):
    d = in0.astype(np.float32) - in1.astype(np.float32)
    t = np.clip(d, s0, s1)
    return (d - imm2 * t) * t


def _oh_ref(in0, in1, s0, s1, imm2):
    raise NotImplementedError


def _register(name, spec, subdim):
    if name in dve_ops._SUB_OPCODE_FOR_NAME:
        return next(o for o in OPS if o.name == name)
    row = dve_ops._CUSTOM_DVE_ROW_BASE + len(OPS)
    assert row < 0x20
    op = DveOp(name, spec, subdim=subdim, uops_sha={})
    for ver in ("v3", "v4"):
        s = DveOpSpec(name=name, opcode=row, uops=lower(spec, ver=ver),
                      rd1_en=_has_src1(spec))
        op.uops_sha[ver] = s.sha(ver)
    OPS.append(op)
    dve_ops._SUB_OPCODE_FOR_NAME[name] = row
    return op


_d = Src0 - Src1
_t = minn(maxx(_d, C0), C1)
SL1_DIFF = _register("SL1_DIFF", Spec(body=(_d - _t * C2) * _t, accum=_add_op,
                                      reference=_sl1_ref), subdim=False)
_pg = PageIdx(C0, C1)
ONEHOT_DOT = _register("ONEHOT_DOT",
                       Spec(body=select(eq(Src1, _pg), Src0, Zero),
                            accum=_add_op, reference=_oh_ref), subdim=True)

FP32 = mybir.dt.float32
BF16 = mybir.dt.bfloat16
AX = mybir.AxisListType
OP = mybir.AluOpType
AF = mybir.ActivationFunctionType

N_CORES = 8
B, K, NS, SEC = 8192, 400, 20, 20
S = K // SEC                    # 20 sections / sample
BC = B // N_CORES               # 1024 samples / core
NCH_KP = BC // 128              # 8 keypoint chunks
SFD = NCH_KP * S                # 160 sections per partition
NCH_CE = 20                     # CE chunks
TOKC = BC * K // NCH_CE         # 20480 tokens / CE chunk
T_CE = TOKC // 128              # 160 tokens / partition (n-major layout)
FFL = TOKC * NS // 100          # 4096 cols in flat [100, .] layout

N_ITER = 2                      # polar Newton iterations

# acc column map
C_LSE = 0                       # 5 cols (groups of 4 chunks; rows 32q+0..4)
C_LY = C_LSE + 5                # 20 cols
C_KP = C_LY + NCH_CE            # 8
C_ROT = C_KP + NCH_KP           # 8
C_CENT = C_ROT + NCH_KP         # 1
NACC = C_CENT + 1


def _emit(ctx, tc, aps):
    nc = tc.nc
    pk, gk, lgn, lgf, lb, ob, out = (aps[k] for k in
                                     ("pk", "gk", "lgn", "lgf", "lb", "ob", "out"))

    pers = ctx.enter_context(tc.tile_pool(name="pers", bufs=1))
    scr = ctx.enter_context(tc.tile_pool(name="scr", bufs=1))
    cep = ctx.enter_context(tc.tile_pool(name="ce", bufs=2))
    psp = ctx.enter_context(tc.tile_pool(name="ps", bufs=1, space="PSUM"))

    acc = pers.tile([128, NACC], FP32, tag="acc", name="acc")
    oneblk = pers.tile([100, 5], BF16, tag="oneblk", name="oneblk")
    nc.sync.dma_start(oneblk[:], ob)
    lnhalf = pers.tile([128, 1], FP32, tag="lnhalf", name="lnhalf")
    nc.gpsimd.memset(lnhalf[:], float(np.log(0.5)))

    # keypoint chunks persist across phase 1 and 3
    pb = [pers.tile([128, 1200], BF16, tag=f"pb{c}", name=f"pb{c}") for c in range(NCH_KP)]
    gb = [pers.tile([128, 1200], BF16, tag=f"gb{c}", name=f"gb{c}") for c in range(NCH_KP)]
    for c in range(NCH_KP):
        nc.sync.dma_start(pb[c][:], pk[c])
        nc.sync.dma_start(gb[c][:], gk[c])

    # ---------------- cross entropy ----------------
    psum = psp.tile([128, FFL], FP32, tag="mm", name="mm")

    def ce_chunk(c):
        lgnc = cep.tile([128, NS * T_CE], BF16, tag="lgn", name="lgn")
        nc.sync.dma_start(lgnc[:], lgn[c])
        lbc = cep.tile([128, T_CE], BF16, tag="lbc", name="lbc")
        nc.sync.dma_start(lbc[:], lb[c])
        lgfc = cep.tile([100, FFL], BF16, tag="lgf", name="lgf")
        nc.sync.dma_start(lgfc[:], lgf[c])

        # l_y: one custom op
        dmp = scr.tile([128, NS * T_CE], BF16, tag="dmp", name="dmp")
        nc.vector._custom_dve(
            ONEHOT_DOT,
            out=dmp[:].rearrange("p (n t) -> p n t", n=NS),
            in0=lgnc[:].rearrange("p (n t) -> p n t", n=NS),
            in1=lbc[:].unsqueeze(1).broadcast_to([128, NS, T_CE]),
            s0=0.0, s1=1.0, accum_out=acc[:, C_LY + c:C_LY + c + 1])

        # lse: exp (scalar) -> block-ones matmul (PE) -> ln on packed PSUM
        ex = lgfc
        nc.scalar.activation(ex[:], lgfc[:], AF.Exp)
        q = c % 4
        for h in range(FFL // 512):
            nc.tensor.matmul(
                psum[32 * q:32 * q + 5, h * 512:(h + 1) * 512],
                oneblk[:], ex[:, h * 512:(h + 1) * 512],
                start=True, stop=True, tile_position=(0, 32 * q))
        if q == 3:
            g = c // 4
            lnd = scr.tile([101, FFL], BF16, tag="lnd", name="lnd")
            nc.scalar.activation(lnd[:], psum[0:101, :], AF.Ln,
                                 accum_out=acc[0:101, C_LSE + g:C_LSE + g + 1])

    for c in range(12):
        ce_chunk(c)

    # ---------------- phase 1: keypoints ----------------
    H = pers.tile([128, 9 * SFD], BF16, tag="H", name="H")
    sp = pers.tile([128, 3 * SFD], BF16, tag="sp", name="sp")
    sg = pers.tile([128, 3 * SFD], BF16, tag="sg", name="sg")

    wk1 = tc.alloc_tile_pool(name="wk1", bufs=2)
    work = wk1
    for c in range(NCH_KP):
        p3 = pb[c][:].rearrange("p (d f) -> p d f", d=3)        # [128,3,400]
        g3 = gb[c][:].rearrange("p (d f) -> p d f", d=3)
        dmp = scr.tile([128, 1200], BF16, tag="dump1200", name="dump1200")
        nc.vector._custom_dve(SL1_DIFF, out=dmp[:], in0=pb[c][:], in1=gb[c][:],
                              s0=-1.0, s1=1.0, imm2=0.5,
                              accum_out=acc[:, C_KP + c:C_KP + c + 1])
        # H products: T[m=(i,j)] = g_i * p_j over (k,s)
        T = work.tile([128, 9 * 400], BF16, tag="hT", name="hT")
        nc.vector.tensor_tensor(
            T[:].rearrange("p (i j f) -> p i j f", i=3, j=3),
            g3.unsqueeze(2).broadcast_to([128, 3, 3, 400]),
            p3.unsqueeze(1).broadcast_to([128, 3, 3, 400]), OP.mult)
        # k-tree: 20 -> 10 -> 5 -> reduce
        T4 = T[:].rearrange("p (m k s) -> p m k s", m=9, k=SEC)
        A1 = work.tile([128, 9 * 10 * S], BF16, tag="hA1", name="hA1")
        A1v = A1[:].rearrange("p (m k s) -> p m k s", m=9, k=10, s=S)
        nc.vector.tensor_tensor(A1v, T4[:, :, 0:10], T4[:, :, 10:20], OP.add)
        A2 = work.tile([128, 9 * 5 * S], BF16, tag="hA2", name="hA2")
        A2v = A2[:].rearrange("p (m k s) -> p m k s", m=9, k=5, s=S)
        nc.vector.tensor_tensor(A2v, A1v[:, :, 0:5], A1v[:, :, 5:10], OP.add)
        D1 = work.tile([128, 9 * 2 * S], BF16, tag="hD1", name="hD1")
        D1v = D1[:].rearrange("p (m k s) -> p m k s", m=9, k=2, s=S)
        nc.vector.tensor_tensor(D1v, A2v[:, :, 0:2], A2v[:, :, 2:4], OP.add)
        D2 = work.tile([128, 9 * S], BF16, tag="hD2", name="hD2")
        D2v = D2[:].rearrange("p (m s) -> p m s", m=9)
        nc.vector.tensor_tensor(D2v, D1v[:, :, 0], D1v[:, :, 1], OP.add)
        nc.vector.tensor_tensor(
            H[:].rearrange("p (m f) -> p m f", m=9)[:, :, c * S:(c + 1) * S],
            D2v, A2v[:, :, 4], OP.add)
        # point sums over k (tree adds, all 2x-mode)
        for src, dst in ((p3, sp), (g3, sg)):
            s4 = src.rearrange("p d (k s) -> p d k s", k=SEC)
            B1 = work.tile([128, 3 * 10 * S], BF16, tag="sB1", name="sB1")
            B1v = B1[:].rearrange("p (d k s) -> p d k s", d=3, k=10, s=S)
            nc.vector.tensor_tensor(B1v, s4[:, :, 0:10], s4[:, :, 10:20], OP.add)
            B2 = work.tile([128, 3 * 5 * S], BF16, tag="sB2", name="sB2")
            B2v = B2[:].rearrange("p (d k s) -> p d k s", d=3, k=5, s=S)
            nc.vector.tensor_tensor(B2v, B1v[:, :, 0:5], B1v[:, :, 5:10], OP.add)
            C1t = work.tile([128, 3 * 2 * S], BF16, tag="sC1", name="sC1")
            C1v = C1t[:].rearrange("p (d k s) -> p d k s", d=3, k=2, s=S)
            nc.vector.tensor_tensor(C1v, B2v[:, :, 0:2], B2v[:, :, 2:4], OP.add)
            C2t = work.tile([128, 3 * S], BF16, tag="sC2", name="sC2")
            C2v = C2t[:].rearrange("p (d s) -> p d s", d=3)
            nc.vector.tensor_tensor(C2v, C1v[:, :, 0], C1v[:, :, 1], OP.add)
            nc.vector.tensor_tensor(
                dst[:].rearrange("p (d f) -> p d f", d=3)[:, :, c * S:(c + 1) * S],
                C2v, B2v[:, :, 4], OP.add)

    wk1.release()
    # center loss: smoothL1((sp-sg)/SEC) over [128, 3*SFD]
    sps = pers.tile([128, 3 * SFD], BF16, tag="sps", name="sps")
    sgs = pers.tile([128, 3 * SFD], BF16, tag="sgs", name="sgs")
    nc.vector.tensor_scalar(sps[:], sp[:], 1.0 / SEC, None, OP.mult)
    nc.vector.tensor_scalar(sgs[:], sg[:], 1.0 / SEC, None, OP.mult)
    dmpc = scr.tile([128, 3 * SFD], BF16, tag="dmpc", name="dmpc")
    nc.vector._custom_dve(SL1_DIFF, out=dmpc[:], in0=sps[:], in1=sgs[:],
                          s0=-1.0, s1=1.0, imm2=0.5,
                          accum_out=acc[:, C_CENT:C_CENT + 1])

    # H -= sg_i * sp_j / SEC
    sp3 = sp[:].rearrange("p (d f) -> p d f", d=3)
    sg3 = sg[:].rearrange("p (d f) -> p d f", d=3)
    M = scr.tile([128, 9 * SFD], BF16, tag="hcM", name="hcM")
    nc.vector.tensor_tensor(
        M[:].rearrange("p (i j f) -> p i j f", i=3, j=3),
        sg3.unsqueeze(2).broadcast_to([128, 3, 3, SFD]),
        sp3.unsqueeze(1).broadcast_to([128, 3, 3, SFD]), OP.mult)
    nc.vector.tensor_scalar(M[:], M[:], 1.0 / SEC, None, OP.mult)
    nc.vector.tensor_tensor(H[:], H[:], M[:], OP.subtract)

    # ---------------- polar decomposition (batched 3x3, bf16) ----------------
    pol = tc.alloc_tile_pool(name="pol", bufs=1)
    A66 = pol.tile([128, 36 * SFD], BF16, tag="A66", name="A66")
    A = A66[:].rearrange("p (a b f) -> p a b f", a=6, b=6)
    X = A[:, 0:3, 0:3]                                  # X lives inside A66
    H4 = H[:].rearrange("p (i j f) -> p i j f", i=3, j=3)
    nc.vector.tensor_copy(X, H4)
    Cf = pol.tile([128, 9 * SFD], BF16, tag="cof", name="cof")
    C3v = Cf[:].rearrange("p (i j f) -> p i j f", i=3, j=3)
    SX = pol.tile([128, 9 * SFD], BF16, tag="sqX", name="sqX")
    det = pol.tile([128, SFD], BF16, tag="det", name="det")
    t160a = pol.tile([128, SFD], FP32, tag="t160a", name="t160a")
    t160b = pol.tile([128, SFD], FP32, tag="t160b", name="t160b")
    adet = pol.tile([128, SFD], FP32, tag="adet", name="adet")
    sgn = pol.tile([128, SFD], BF16, tag="sgn", name="sgn")
    nx2 = pol.tile([128, SFD], BF16, tag="nx2", name="nx2")
    nc2_ = pol.tile([128, SFD], BF16, tag="nc2", name="nc2")
    zln = pol.tile([128, SFD], FP32, tag="zln", name="zln")
    hz = pol.tile([128, SFD], BF16, tag="hz", name="hz")
    wz = pol.tile([128, SFD], BF16, tag="wz", name="wz")
    wf = pol.tile([128, SFD], FP32, tag="wf", name="wf")

    def frob(dst, src4):
        sxw = SX[:].rearrange("p (i j f) -> p i j f", i=3, j=3)
        nc.vector.tensor_tensor(sxw, src4, src4, OP.mult)
        sx = SX[:].rearrange("p (m f) -> p m f", m=9)
        q1 = pol.tile([128, 4 * SFD], BF16, tag="fq1", name="fq1")
        q1v = q1[:].rearrange("p (m f) -> p m f", m=4)
        nc.vector.tensor_tensor(q1v, sx[:, 0:4], sx[:, 4:8], OP.add)
        q2 = pol.tile([128, 2 * SFD], BF16, tag="fq2", name="fq2")
        q2v = q2[:].rearrange("p (m f) -> p m f", m=2)
        nc.vector.tensor_tensor(q2v, q1v[:, 0:2], q1v[:, 2:4], OP.add)
        nc.vector.tensor_tensor(dst.unsqueeze(1), q2v[:, 0:1], q2v[:, 1:2], OP.add)
        nc.vector.tensor_tensor(dst, dst, sx[:, 8], OP.add)

    for it in range(N_ITER):
        # duplicate X -> A66 quadrants
        nc.vector.tensor_copy(A[:, 0:3, 3:6], X)
        nc.vector.tensor_copy(A[:, 3:6, :], A[:, 0:3, :])
        # cofactors: C[i][j] = A[i+1][j+1]A[i+2][j+2] - A[i+1][j+2]A[i+2][j+1]
        T1 = pol.tile([128, 9 * SFD], BF16, tag="ct1", name="ct1")
        nc.vector.tensor_tensor(
            T1[:].rearrange("p (i j f) -> p i j f", i=3, j=3),
            A[:, 1:4, 1:4], A[:, 2:5, 2:5], OP.mult)
        T2 = pol.tile([128, 9 * SFD], BF16, tag="ct2", name="ct2")
        nc.vector.tensor_tensor(
            T2[:].rearrange("p (i j f) -> p i j f", i=3, j=3),
            A[:, 1:4, 2:5], A[:, 2:5, 1:4], OP.mult)
        nc.vector.tensor_tensor(Cf[:], T1[:], T2[:], OP.subtract)
        # det = sum_j X[0][j] * C[0][j]
        P0 = pol.tile([128, 3 * SFD], BF16, tag="dp0", name="dp0")
        P0v = P0[:].rearrange("p (j f) -> p j f", j=3)
        nc.vector.tensor_tensor(P0v, X[:, 0], C3v[:, 0], OP.mult)
        nc.vector.tensor_tensor(det[:].unsqueeze(1), P0v[:, 0:1], P0v[:, 1:2], OP.add)
        nc.vector.tensor_tensor(det[:], det[:], P0v[:, 2], OP.add)
        # guards + zeta = exp(0.25 ln(nC2/nX2) - 0.5 ln|det|)
        frob(nx2[:], X)
        frob(nc2_[:], C3v)
        nc.vector.tensor_scalar(nx2[:], nx2[:], 1e-12, None, OP.max)
        nc.vector.tensor_scalar(nc2_[:], nc2_[:], 1e-12, None, OP.max)
        nc.scalar.activation(adet[:], det[:], AF.Abs)
        nc.vector.tensor_scalar(adet[:], adet[:], 1e-6, None, OP.max)
        nc.vector.tensor_scalar(sgn[:], det[:], 0.0, None, OP.is_ge)
        nc.vector.tensor_scalar(sgn[:], sgn[:], 2.0, -1.0, OP.mult, OP.add)
        nc.scalar.activation(t160a[:], nc2_[:], AF.Ln)
        nc.scalar.activation(t160b[:], nx2[:], AF.Ln)
        nc.vector.tensor_tensor(t160a[:], t160a[:], t160b[:], OP.subtract)
        nc.scalar.activation(t160b[:], adet[:], AF.Ln)
        nc.vector.tensor_scalar(t160b[:], t160b[:], -2.0, None, OP.mult)
        nc.vector.tensor_tensor(zln[:], t160a[:], t160b[:], OP.add)
        nc.vector.tensor_scalar(zln[:], zln[:], 0.25, None, OP.mult)
        # hz = 0.5*zeta = exp(zln + ln 0.5); w = 0.5*sgn/(zeta*|det|)
        nc.scalar.activation(hz[:], zln[:], AF.Exp, bias=lnhalf[:])
        nc.scalar.activation(t160a[:], zln[:], AF.Exp)
        nc.vector.tensor_tensor(t160b[:], t160a[:], adet[:], OP.mult)
        nc.vector.tensor_scalar(t160b[:], t160b[:], 2.0, None, OP.mult)
        nc.vector.reciprocal_approx_fast(wf[:], t160b[:])
        nc.vector.tensor_tensor(wz[:], wf[:], sgn[:], OP.mult)
        # X = X*hz + C*w  (broadcast over 9 components)
        hzb = hz[:].unsqueeze(1).unsqueeze(1).broadcast_to([128, 3, 3, SFD])
        wzb = wz[:].unsqueeze(1).unsqueeze(1).broadcast_to([128, 3, 3, SFD])
        U1 = pol.tile([128, 9 * SFD], BF16, tag="u1", name="u1")
        U1v = U1[:].rearrange("p (i j f) -> p i j f", i=3, j=3)
        nc.vector.tensor_tensor(U1v, X, hzb, OP.mult)
        U2 = pol.tile([128, 9 * SFD], BF16, tag="u2", name="u2")
        U2v = U2[:].rearrange("p (i j f) -> p i j f", i=3, j=3)
        nc.vector.tensor_tensor(U2v, C3v, wzb, OP.mult)
        nc.vector.tensor_tensor(X, U1v, U2v, OP.add)

    # Newton-Schulz polish: R = X (1.5 I - 0.5 X^T X)
    Y = pol.tile([128, 9 * SFD], BF16, tag="Y", name="Y")
    Yv = Y[:].rearrange("p (i j f) -> p i j f", i=3, j=3)
    Tk = pol.tile([128, 9 * SFD], BF16, tag="Tk", name="Tk")
    Tkv = Tk[:].rearrange("p (i j f) -> p i j f", i=3, j=3)
    for k in range(3):
        xk = A[:, k, 0:3]                               # [128, 3, SFD] = X[k][*]
        dst = Yv if k == 0 else Tkv
        nc.vector.tensor_tensor(
            dst, xk.unsqueeze(2).broadcast_to([128, 3, 3, SFD]),
            xk.unsqueeze(1).broadcast_to([128, 3, 3, SFD]), OP.mult)
        if k:
            nc.vector.tensor_tensor(Y[:], Y[:], Tk[:], OP.add)
    W = pol.tile([128, 9 * SFD], BF16, tag="W", name="W")
    nc.vector.tensor_scalar(W[:], Y[:], -0.5, None, OP.mult)
    Wv = W[:].rearrange("p (m f) -> p m f", m=9)
    for m in (0, 4, 8):
        nc.vector.tensor_scalar(Wv[:, m], Wv[:, m], 1.5, None, OP.add)
    R = pers.tile([128, 9 * SFD], BF16, tag="R", name="R")
    Rv = R[:].rearrange("p (i j f) -> p i j f", i=3, j=3)
    Wv4 = W[:].rearrange("p (k j f) -> p k j f", k=3, j=3)
    for k in range(3):
        dst = Rv if k == 0 else Tkv
        nc.vector.tensor_tensor(
            dst, A[:, 0:3, k].unsqueeze(2).broadcast_to([128, 3, 3, SFD]),
            Wv4[:, k].unsqueeze(1).broadcast_to([128, 3, 3, SFD]), OP.mult)
        if k:
            nc.vector.tensor_tensor(R[:], R[:], Tk[:], OP.add)

    # v_j = (sum_i sp_i R_ij - sg_j)/SEC
    v = pers.tile([128, 3 * SFD], BF16, tag="v", name="v")
    vv = v[:].rearrange("p (j f) -> p j f", j=3)
    Pv = pol.tile([128, 9 * SFD], BF16, tag="Pv", name="Pv")
    Pvv = Pv[:].rearrange("p (i j f) -> p i j f", i=3, j=3)
    nc.vector.tensor_tensor(
        Pvv, sp3.unsqueeze(2).broadcast_to([128, 3, 3, SFD]), Rv, OP.mult)
    nc.vector.tensor_tensor(vv, Pvv[:, 0], Pvv[:, 1], OP.add)
    nc.vector.tensor_tensor(vv, vv, Pvv[:, 2], OP.add)
    nc.vector.tensor_tensor(vv, vv, sg3, OP.subtract)
    nc.vector.tensor_scalar(v[:], v[:], 1.0 / SEC, None, OP.mult)

    pol.release()
    # ---------------- phase 3: rotation residual ----------------
    wk3 = tc.alloc_tile_pool(name="wk3", bufs=2)
    work = wk3
    for c in range(NCH_KP):
        p3 = pb[c][:].rearrange("p (d k s) -> p d k s", d=3, k=SEC)
        g3 = gb[c][:].rearrange("p (d k s) -> p d k s", d=3, k=SEC)
        Rc = Rv[:, :, :, c * S:(c + 1) * S]              # [128,3,3,S]
        vc = vv[:, :, c * S:(c + 1) * S]                 # [128,3,S]
        T0 = work.tile([128, 1200], BF16, tag="r0", name="r0")
        T1_ = work.tile([128, 1200], BF16, tag="r1", name="r1")
        T2_ = work.tile([128, 1200], BF16, tag="r2", name="r2")
        for i, Td in enumerate((T0, T1_, T2_)):
            nc.vector.tensor_tensor(
                Td[:].rearrange("p (j k s) -> p j k s", j=3, k=SEC),
                p3[:, i].unsqueeze(1).broadcast_to([128, 3, SEC, S]),
                Rc[:, i].unsqueeze(2).broadcast_to([128, 3, SEC, S]), OP.mult)
        nc.gpsimd.tensor_tensor(T0[:], T0[:], T1_[:], OP.add)
        Bt = work.tile([128, 1200], BF16, tag="rB", name="rB")
        nc.gpsimd.tensor_tensor(
            Bt[:].rearrange("p (j k s) -> p j k s", j=3, k=SEC),
            g3, vc.unsqueeze(2).broadcast_to([128, 3, SEC, S]), OP.add)
        nc.vector.tensor_tensor(Bt[:], Bt[:], T2_[:], OP.subtract)
        dmp = scr.tile([128, 1200], BF16, tag="dump1200", name="dump1200")
        nc.vector._custom_dve(SL1_DIFF, out=dmp[:], in0=T0[:], in1=Bt[:],
                              s0=-1.0, s1=1.0, imm2=0.5,
                              accum_out=acc[:, C_ROT + c:C_ROT + c + 1])

    wk3.release()
    for c in range(12, NCH_CE):
        ce_chunk(c)
    nc.sync.dma_start(out[:], acc[:])


_CACHE = {}


def _build():
    if "nc" in _CACHE:
        return _CACHE["nc"]
    nc = bacc.Bacc("TRN2", target_bir_lowering=False, debug=False,
                   enable_asserts=False, num_devices=N_CORES)
    aps = {
        "pk": nc.dram_tensor("pk", [NCH_KP, 128, 1200], BF16, kind="ExternalInput").ap(),
        "gk": nc.dram_tensor("gk", [NCH_KP, 128, 1200], BF16, kind="ExternalInput").ap(),
        "lgn": nc.dram_tensor("lgn", [NCH_CE, 128, NS * T_CE], BF16, kind="ExternalInput").ap(),
        "lgf": nc.dram_tensor("lgf", [NCH_CE, 100, FFL], BF16, kind="ExternalInput").ap(),
        "lb": nc.dram_tensor("lb", [NCH_CE, 128, T_CE], BF16, kind="ExternalInput").ap(),
        "ob": nc.dram_tensor("ob", [100, 5], BF16, kind="ExternalInput").ap(),
        "out": nc.dram_tensor("out", [128, NACC], FP32, kind="ExternalOutput").ap(),
    }
    with tile.TileContext(nc) as tc:
        with ExitStack() as ctx:
            with nc.allow_low_precision(reason="bf16 5-term tree reduces; validated"):
                _emit(ctx, tc, aps)
    nc.compile()
    _CACHE["nc"] = nc
    return nc


def _shard_inputs(pred_keypoints, gt_keypoints, pred_section_logits, gt_section_label):
    bf = ml_dtypes.bfloat16
    pkh = np.asarray(pred_keypoints, dtype=np.float32).reshape(
        N_CORES, NCH_KP, 128, S, SEC, 3).transpose(0, 1, 2, 5, 4, 3)
    pkh = np.ascontiguousarray(pkh).reshape(N_CORES, NCH_KP, 128, 1200).astype(bf)
    gkh = np.asarray(gt_keypoints, dtype=np.float32).reshape(
        N_CORES, NCH_KP, 128, S, SEC, 3).transpose(0, 1, 2, 5, 4, 3)
    gkh = np.ascontiguousarray(gkh).reshape(N_CORES, NCH_KP, 128, 1200).astype(bf)
    lg32 = np.asarray(pred_section_logits, dtype=np.float32)
    lgnh = lg32.reshape(N_CORES, NCH_CE, 128, T_CE, NS).transpose(0, 1, 2, 4, 3)
    lgnh = np.ascontiguousarray(lgnh).reshape(N_CORES, NCH_CE, 128, NS * T_CE).astype(bf)
    lgfh = lg32.reshape(N_CORES, NCH_CE, FFL, 100).transpose(0, 1, 3, 2)
    lgfh = np.ascontiguousarray(lgfh).astype(bf)
    lbh = np.asarray(gt_section_label).reshape(N_CORES, NCH_CE, 128, T_CE).astype(bf)
    ob = np.zeros((100, 5), dtype=np.float32)
    for g in range(5):
        ob[g * 20:(g + 1) * 20, g] = 1.0
    ob = ob.astype(bf)
    return [{"pk": pkh[i], "gk": gkh[i], "lgn": lgnh[i], "lgf": lgfh[i],
             "lb": lbh[i], "ob": ob} for i in range(N_CORES)]


def combine_accs(accs):
    tot = np.zeros(NACC, dtype=np.float64)
    lse = 0.0
    for a in accs:
        a64 = a.astype(np.float64)
        tot += a64.sum(axis=0)
        for g in range(5):
            col = a64[:, C_LSE + g]
            for q in range(4):
                lse += col[32 * q:32 * q + 5].sum()
    ly = tot[C_LY:C_LY + NCH_CE].sum()
    kp = tot[C_KP:C_KP + NCH_KP].sum()
    rot = tot[C_ROT:C_ROT + NCH_KP].sum()
    cent = tot[C_CENT]
    total = (1.0 * (lse - ly) / (B * K)
             + 4.0 * kp / (B * K * 3)
             + 5.0 * rot / (B * K * 3)
             + 6.0 * cent / (B * S * 3))
    return np.float32(total)


def kernel(**inputs) -> np.ndarray:
    nc = _build()
    in_maps = _shard_inputs(**inputs)
    res = run_bass_kernel_spmd(nc, in_maps, list(range(N_CORES))).results
    return combine_accs([res[i]["out"] for i in range(N_CORES)])
